# revision 43
# baseline (speedup 1.0000x reference)
"""BitNet-style attention block (ternary-quantized QKV/proj) on 8 Trainium2 cores.

Strategy: data-parallel over batch (16 batches -> 2 per core, no collectives).

v5 (462us v3 baseline -> ~390us fast-clock):
  - Q/K generation runs in fp8e4 DoubleRow (K=256 per pass, measured at
    bf16's per-instruction rate -> half the qkgen PE time). Ternary
    weights are exact in fp8; only x quantizes (~3% el-wise), which the
    softmax damps to ~5e-3 output error. V stays bf16 (fp8 V alone
    costs ~2.6e-2 rel error - over the 2e-2 budget).
  - ACT is exp-only plus the 4 per-pair [65,512] PSUM->SBUF AV
    evacuations (its idle boundary window); exp uses an immediate scale
    (Q pre-scaled by SCALE*s^2 at evac), 1079ns/[128,1024] measured.
  - All PSUM evacuations (qk/vgen/proj) on DVE; softmax epilogue per
    (head, query-half): l-row copy -> reciprocal_approx_fast (DVE) ->
    partition_broadcast (GpSimd) -> multiply (DVE). GpSimd runs ONLY
    partition_broadcast steady-state: its other ops live in a different
    Q7 library and each switch costs a ~6us LIBRARY_RELOAD stall.
  - Emission pipeline per kb: ST(kb,0) exp(kb,0) ST(kb,1) exp(kb,1)
    AV(kb-1,1) [epilogue part / filler unit] AV(kb,0). The trailing
    AV(7,1) + avsb evac land at the next pair's kb0 and the 4
    normalize chains spread over its kb1-4 (cross-pair pipelining,
    no DVE head-of-line blocking of the psum-slot-recycling evacs).
  - Filler units (3-6 matmuls + evac) are placed per (pair, kb) by an
    explicit JIT table; kb7 gets a unit so the boundary kb is not
    PE-light. PSUM: 2x st2 [128,1024] + 4x AV accumulators = all 8
    banks; filler units steal exp-freed st2 slots.
  - Inputs land as per-partition-contiguous DMAs (128 descriptors
    each) with separate tiles per piece so the first qkgen waits only
    on its own 0.9MB; priority order x8(b0) / wq blocks 0+6 first.
    Output DMAs alternate sync/scalar queues; proj emits per
    512-token half; the last pair's qi=1 normalize chains overlap the
    tail's qi=0 proj units.
  - Device clock varies run-to-run (~391 vs ~460us for the same NEFF);
    compare kernels by min over a few runs.
"""

import os
import sys

import ml_dtypes
import numpy as np

for _p in ("/opt/trn_rl_repo", "/root/.axon_site/_ro/trn_rl_repo"):
    if os.path.isdir(_p) and _p not in sys.path:
        sys.path.insert(0, _p)

import concourse.bass as bass
import concourse.mybir as mybir
import concourse.tile as tile
from concourse import bacc
from concourse.bass_utils import run_bass_kernel_spmd

B, N, C, H = 16, 1024, 768, 12
HD = C // H                    # 64
SCALE = float(HD ** -0.5)      # 0.125
EPS = 1e-5
NCORES = 8
BPC = B // NCORES              # 2 batches per core
T = BPC * N                    # 2048 tokens per core
P = 128
CB = C // P                    # 6 c-blocks of 128
MQK = 2 * CB                   # 12 d-blocks covering Q and K
HP = H // 2                    # 6 head pairs
KB = N // P                    # 8 key blocks per batch
F32 = mybir.dt.float32
BF16 = mybir.dt.bfloat16
FP8 = mybir.dt.float8e4
DR = mybir.MatmulPerfMode.DoubleRow
AF = mybir.ActivationFunctionType
ALU = mybir.AluOpType

_CACHED_NC = None
_DEBUG = False


def _split_drain_waits(nc):
    """The walrus build in this container accepts only one sync-wait per
    instruction; move extra waits onto preceding single-wait NoOps on the
    same engine (in-order queues make this semantics-preserving)."""
    for fn in nc.m.functions:
        for bb in fn.blocks:
            insts = bb.instructions
            i = 0
            while i < len(insts):
                inst = insts[i]
                si = getattr(inst, "sync_info", None)
                if (
                    si is not None
                    and si.on_wait is not None
                    and len(si.on_wait) > 1
                    and not type(inst).__name__.startswith("InstDMA")
                ):
                    waits = list(si.on_wait)
                    for j, w in enumerate(waits[:-1]):
                        nop = mybir.InstNoOp(
                            name=f"{inst.name}-prewait-{j}", ins=[], outs=[]
                        )
                        nop.engine = inst.engine
                        nop.sync_info = mybir.SyncInfo(on_wait=[w], on_update=[])
                        insts.insert(i, nop)
                        i += 1
                    inst.sync_info = mybir.SyncInfo(
                        on_wait=[waits[-1]], on_update=list(si.on_update)
                    )
                i += 1


def _build_nc():
    nc = bacc.Bacc(None)

    xT = nc.dram_tensor("xT", [P, 2, 6 * N], FP8, kind="ExternalInput")
    xT16 = nc.dram_tensor("xT16", [P, 2, CB * N], BF16, kind="ExternalInput")
    wq16 = nc.dram_tensor("wq16", [P, 6 * 2 * C], FP8, kind="ExternalInput")
    wv16 = nc.dram_tensor("wv16", [P, CB, C], BF16, kind="ExternalInput")
    wp16 = nc.dram_tensor("wp16", [P, CB, C], BF16, kind="ExternalInput")
    bp = nc.dram_tensor("bp", [C], F32, kind="ExternalInput")
    sq = nc.dram_tensor("sq", [1, 2], F32, kind="ExternalInput")  # [s, SCALE*s^2]
    sp = nc.dram_tensor("sp", [1, 1], F32, kind="ExternalInput")  # [s]
    cz = nc.dram_tensor("cz", [2, N], BF16, kind="ExternalInput")  # row0=0, row1=1
    yT = nc.dram_tensor("yT", [CB, P, T], F32, kind="ExternalOutput")
    if _DEBUG:
        qk_dbg = nc.dram_tensor("qk_dbg", [P, MQK, T], BF16, kind="ExternalOutput")
        va_dbg = nc.dram_tensor(
            "va_dbg", [P, 2 * KB, H, HD + 1], BF16, kind="ExternalOutput"
        )
        out_dbg = nc.dram_tensor("out_dbg", [P, CB, T], BF16, kind="ExternalOutput")
        l_dbg = nc.dram_tensor("l_dbg", [4, 512], F32, kind="ExternalOutput")
        li_dbg = nc.dram_tensor("li_dbg", [4, 512], F32, kind="ExternalOutput")
        bc_dbg = nc.dram_tensor("bc_dbg", [4, HD, 512], F32, kind="ExternalOutput")
        am_dbg = nc.dram_tensor("am_dbg", [4, HD, 512], F32, kind="ExternalOutput")

    with tile.TileContext(nc) as tc:
        with (
            tc.tile_pool(name="constp", bufs=1) as constp,
            tc.tile_pool(name="xp", bufs=2) as xp,
            tc.tile_pool(name="wqab", bufs=2) as wqab,
            tc.tile_pool(name="wqcd", bufs=2) as wqcd,
            tc.tile_pool(name="wvp", bufs=1) as wvp,
            tc.tile_pool(name="wpp", bufs=1) as wpp,
            tc.tile_pool(name="vaugp", bufs=1) as vaugp,
            tc.tile_pool(name="qksp", bufs=1) as qksp,
            tc.tile_pool(name="outp", bufs=1) as outp,
            tc.tile_pool(name="ep", bufs=4) as ep,
            tc.tile_pool(name="linvp", bufs=2) as linvp,
            tc.tile_pool(name="bcp", bufs=2) as bcp,
            tc.tile_pool(name="avsp", bufs=4) as avsp,
            tc.tile_pool(name="ystp", bufs=2) as ystp,
            tc.tile_pool(name="psp", bufs=2, space="PSUM") as psp,   # [P,1024] 2-bank
            tc.tile_pool(name="avp", bufs=4, space="PSUM") as avp,   # [65,512] 1-bank
        ):
            # ---- scalars / bias / ones ----
            sqb = constp.tile([P, 2], F32, tag="sqb")
            spb = constp.tile([P, 1], F32, tag="spb")
            b_sb = constp.tile([P, CB], F32, tag="b_sb")
            ones_col = constp.tile([P, 1], BF16, tag="ones_col")

            # ---- inputs, ordered so pair-0 work starts ASAP ----
            # scalar hwdge queue: x by token halves (first half feeds all of
            # batch 0); sync queue: wq d-blocks 0 and 6 (pair-0 Q/K), then
            # the rest; gpsimd queue: wv/wp in parallel.
            # x / wq pieces are separate tiles: dependency domains match
            # DMA granularity exactly (first matmul can't get stuck behind
            # the last weight transfer), and every transfer is one
            # contiguous chunk per partition (128 descriptors, cheap gen).
            # Queue order = priority: scalar queue x0 -> wv -> x1 -> wp
            # (wv feeds the startup vgen units; x1/wp are needed much
            # later), sync queue wq blocks 0/6 first.
            # fp8e4 DoubleRow: partition p carries feature f = pass*256 +
            # kt*128 + p (kt in {0,1}); every qkgen/vgen matmul contracts
            # 256 features per pass at bf16 streaming rate (measured: same
            # ~215ns per 512-free instruction, half the instruction count)
            xs_t = [xp.tile([P, 6 * N], FP8, tag="x8", name=f"x8{q}")
                    for q in range(2)]
            xs16 = [xp.tile([P, CB * N], BF16, tag="x16", name=f"x16{q}")
                    for q in range(2)]
            wq_a = wqab.tile([P, 6 * P], FP8, tag="wqa")
            wq_b = wqab.tile([P, 6 * P], FP8, tag="wqb")
            wq_c = wqcd.tile([P, 6 * (C - P)], FP8, tag="wqc")
            wq_d = wqcd.tile([P, 6 * (C - P)], FP8, tag="wqd")
            wv_q = wvp.tile([P, CB, C], BF16, tag="wv")
            wp_q = wpp.tile([P, CB, C], BF16, tag="wp")
            nc.sync.dma_start(wq_a[:], wq16[:, 0 : 6 * P])
            nc.sync.dma_start(wq_b[:], wq16[:, 6 * P : 12 * P])
            nc.sync.dma_start(sqb[:], sq[:, :].to_broadcast([P, 2]))
            nc.sync.dma_start(ones_col[:], cz[1:2, 0:1].to_broadcast([P, 1]))
            nc.sync.dma_start(spb[:], sp[:, :].to_broadcast([P, 1]))
            nc.sync.dma_start(b_sb[:], bp[:].rearrange("(cb p) -> p cb", p=P))
            nc.scalar.dma_start(xs_t[0][:], xT[:, 0, :])
            nc.scalar.dma_start(xs16[0][:], xT16[:, 0, :])
            nc.scalar.dma_start(wv_q[:].rearrange("p a b -> p (a b)"),
                                wv16[:, :, :].rearrange("p a b -> p (a b)"))
            nc.sync.dma_start(wq_c[:], wq16[:, 12 * P : 12 * P + 6 * (C - P)])
            nc.sync.dma_start(
                wq_d[:], wq16[:, 12 * P + 6 * (C - P) : 12 * C]
            )
            nc.scalar.dma_start(xs_t[1][:], xT[:, 1, :])
            nc.sync.dma_start(xs16[1][:], xT16[:, 1, :])
            nc.sync.dma_start(wp_q[:].rearrange("p a b -> p (a b)"),
                                wp16[:, :, :].rearrange("p a b -> p (a b)"))

            def x16_v(q, ci, lo, hi):
                return xs16[q][:].rearrange("p (c n) -> p c n", c=CB)[:, ci, lo:hi]

            def x_v(q, ps_, lo, hi):
                # [128, 2, hi-lo] rhs/lhsT: (partition, ktile, token)
                v = xs_t[q][:].rearrange("p (a k n) -> p a k n", a=3, k=2)
                return v[:, ps_, :, lo:hi]

            def wq_blk(mi, ps_):
                # [128, 2, 128] lhsT for d-block mi, contraction pass ps_
                if mi == 0:
                    t, off = wq_a, 0
                elif mi == CB:
                    t, off = wq_b, 0
                elif mi < CB:
                    t, off = wq_c, mi - 1
                else:
                    t, off = wq_d, mi - CB - 1
                v = t[:].rearrange("p (a k m) -> p a k m", a=3, k=2)
                return v[:, ps_, :, off * P : (off + 1) * P]

            # ---- V-augmented tile ones column ----
            v_aug = vaugp.tile([P, 2 * KB, H, HD + 1], BF16, tag="vaug")
            nc.vector.tensor_copy(
                v_aug[:, :, :, HD : HD + 1],
                ones_col[:, None, :].to_broadcast([P, 2 * KB, H, 1]),
            )

            qksb = qksp.tile([P, MQK, T], BF16, tag="qksb")
            outT = outp.tile([P, CB, T], BF16, tag="outT")

            # ---------------- filler units (one psp slot each) ----------------

            def u_qkgen(mi, qc, half):
                """Q/K d-block mi, one 512-token half of batch qc (3 fp8
                DoubleRow matmuls, K=256 each). Q blocks (mi<CB) pre-scaled
                by SCALE*s^2 so exp runs with an immediate scale."""
                ps = psp.tile([P, 512], F32, tag="ps", name=f"qg{mi}_{qc}_{half}")
                for ki in range(3):
                    nc.tensor.matmul(
                        ps[:],
                        wq_blk(mi, ki),
                        x_v(qc, ki, half * 512, (half + 1) * 512),
                        start=(ki == 0),
                        stop=(ki == 2),
                        perf_mode=DR,
                    )
                dst = qksb[:, mi, qc * N + half * 512 : qc * N + (half + 1) * 512]
                if mi < CB:
                    nc.vector.tensor_scalar_mul(dst, ps[:], sqb[:, 1:2])
                else:
                    nc.vector.tensor_copy(dst, ps[:])

            def u_vgen(tb, nch):
                """V (scaled by s) for one 128-token block, heads
                nch*6..nch*6+5 (6 bf16 matmuls)."""
                ps = psp.tile([P, 512], F32, tag="ps", name=f"vg{tb}_{nch}")
                for ci in range(CB):
                    nc.tensor.matmul(
                        ps[:, 0:384],
                        x16_v(tb // KB, ci, (tb % KB) * P, (tb % KB + 1) * P),
                        wv_q[:, ci, nch * 384 : (nch + 1) * 384],
                        start=(ci == 0),
                        stop=(ci == CB - 1),
                    )
                nc.vector.tensor_scalar_mul(
                    v_aug[:, tb, nch * 6 : (nch + 1) * 6, 0:HD],
                    ps[:, 0:384].rearrange("p (h d) -> p h d", d=HD),
                    sqb[:, 0:1],
                )

            def u_proj(b, co, half, q=None):
                """proj output block co, one 512-token half of batch b
                (6 matmuls; bias+scale on DVE; result DMA on queue q)."""
                ps = psp.tile([P, 512], F32, tag="ps", name=f"pj{b}_{co}_{half}")
                for ci in range(CB):
                    nc.tensor.matmul(
                        ps[:],
                        wp_q[:, ci, co * P : (co + 1) * P],
                        outT[:, ci, b * N + half * 512 : b * N + (half + 1) * 512],
                        start=(ci == 0),
                        stop=(ci == CB - 1),
                    )
                yst = ystp.tile([P, 512], F32, tag="evac",
                                name=f"ye{b}_{co}_{half}")
                nc.vector.tensor_scalar(
                    yst[:], ps[:], spb[:, 0:1], b_sb[:, co : co + 1],
                    ALU.mult, ALU.add,
                )
                (q or nc.sync).dma_start(
                    yT[co, :, b * N + half * 512 : b * N + (half + 1) * 512],
                    yst[:],
                )

            # ---------------- attention ----------------

            def emit_epilogue_chain(b, hp, hh, qi, srcl, srcm, j):
                """one (hh, qi) softmax-normalize chain: l-row -> partition 0
                (DVE), reciprocal (DVE custom), partition-broadcast (GpSimd,
                the only op class it runs steady-state: keeps the attn
                library resident, no Q7 library reloads), multiply (DVE)."""
                lrow = linvp.tile([1, 512], F32, tag="lrow", name=f"lr{j}")
                nc.vector.tensor_copy(lrow[:], srcl)
                linv = linvp.tile([1, 512], F32, tag="linv", name=f"li{j}")
                nc.vector.reciprocal_approx_fast(linv[:], lrow[:])
                bc = bcp.tile([HD, 512], F32, tag="bc", name=f"bc{j}")
                nc.gpsimd.partition_broadcast(bc[:], linv[0:1, :], channels=HD)
                nc.vector.tensor_mul(
                    out=outT[
                        hh * HD : (hh + 1) * HD,
                        hp,
                        b * N + qi * 512 : b * N + (qi + 1) * 512,
                    ],
                    in0=srcm,
                    in1=bc[:],
                )

            def emit_attn(b, hp, units, unit_kbs, pending, last):
                """attention for batch b, heads 2hp/2hp+1. `units`: filler
                closures; `unit_kbs`: kb step for each. `pending`: list of
                closures finishing the PREVIOUS pair (trailing AV + epilogue
                chains), spread over kb0..kb3 so their DVE work never
                head-of-line-blocks the unit evacs that recycle psum slots.
                Returns this pair's pending list (or emits inline if last)."""
                avs = {}
                e2s = {}

                def emit_st(kb, hh):
                    roff = hh * HD
                    st2 = psp.tile([P, 1024], F32, tag="ps", name=f"st{hh}")
                    for half in range(2):
                        nc.tensor.matmul(
                            st2[:, half * 512 : (half + 1) * 512],
                            qksb[roff : roff + HD, CB + hp,
                                 b * N + kb * P : b * N + (kb + 1) * P],
                            qksb[roff : roff + HD, hp,
                                 b * N + half * 512 : b * N + (half + 1) * 512],
                            start=True,
                            stop=True,
                        )
                    e2 = ep.tile([P, 1024], BF16, tag="e2", name=f"e{hh}")
                    nc.scalar.activation(e2[:], st2[:], AF.Exp, bias=0.0)
                    e2s[(kb, hh)] = e2

                def emit_av(kb, hh):
                    h = 2 * hp + hh
                    e2 = e2s.pop((kb, hh))
                    if kb == 0:
                        # lazy alloc: the previous pair's trailing accesses to
                        # these pool slots must be emitted before the slots
                        # are recycled
                        avs[hh] = [
                            avp.tile(
                                [HD + 1, 512], F32, tag="av", name=f"av{hh}{qi}"
                            )
                            for qi in range(2)
                        ]
                    for qi in range(2):
                        nc.tensor.matmul(
                            avs[hh][qi][:],
                            v_aug[:, b * KB + kb, h, :],
                            e2[:, qi * 512 : (qi + 1) * 512],
                            start=(kb == 0),
                            stop=(kb == KB - 1),
                        )

                ui = 0
                pending = list(pending or [])
                for kb in range(KB):
                    emit_st(kb, 0)
                    emit_st(kb, 1)
                    if kb > 0:
                        emit_av(kb - 1, 1)
                    if kb < len(pending):
                        pending[kb]()
                    while ui < len(units) and unit_kbs[ui] <= kb:
                        units[ui]()
                        ui += 1
                    emit_av(kb, 0)
                while ui < len(units):
                    units[ui]()
                    ui += 1

                # hh0 finished accumulating (stop at kb=KB-1): evacuate its
                # PSUM promptly unless this is the final pair
                if not last:
                    avsb = {}
                    for qi in range(2):
                        t = avsp.tile([HD + 1, 512], F32, tag="avsb",
                                      name=f"ab0{qi}")
                        nc.scalar.activation(t[:], avs[0][qi][:], AF.Copy, bias=0.0)
                        avsb[(0, qi)] = t

                    def part0():
                        emit_av(KB - 1, 1)
                        for qi in range(2):
                            t = avsp.tile([HD + 1, 512], F32, tag="avsb",
                                          name=f"ab1{qi}")
                            nc.scalar.activation(
                                t[:], avs[1][qi][:], AF.Copy, bias=0.0
                            )
                            avsb[(1, qi)] = t

                    def chain(j):
                        hh, qi = j // 2, j % 2
                        emit_epilogue_chain(
                            b, hp, hh, qi,
                            avsb[(hh, qi)][HD : HD + 1, :],
                            avsb[(hh, qi)][0:HD, :],
                            j,
                        )

                    return [part0] + [
                        (lambda j=j: chain(j)) for j in range(4)
                    ]

                # final pair: trailing AV + epilogue straight from PSUM.
                # qi=0 chains now; qi=1 chains deferred so the tail's first
                # proj half-units (which only need qi=0 of outT) overlap them
                emit_av(KB - 1, 1)
                for hh in range(2):
                    emit_epilogue_chain(
                        b, hp, hh, 0,
                        avs[hh][0][HD : HD + 1, :],
                        avs[hh][0][0:HD, :],
                        hh * 2,
                    )

                def rest():
                    for hh in range(2):
                        emit_epilogue_chain(
                            b, hp, hh, 1,
                            avs[hh][1][HD : HD + 1, :],
                            avs[hh][1][0:HD, :],
                            hh * 2 + 1,
                        )

                return rest

            # ---------------- schedule ----------------
            # startup: pair-0 Q/K only; V blocks stream in as pair-0 units
            # (the exp chain starts ~8us earlier than a vgen pre-burst)
            u_qkgen(0, 0, 0)
            u_qkgen(0, 0, 1)
            u_qkgen(CB, 0, 0)
            u_qkgen(CB, 0, 1)

            # per-pair filler unit lists (just-in-time: a pair's Q/K units
            # land >=1 pair ahead; vgen b1 lands >=2 pairs ahead; proj b0
            # waits for the b0 pair-5 epilogue which lands at b1-pair0 kb0)
            qk = u_qkgen
            vg = u_vgen
            pj = u_proj

            def QK(mi, qc, kb):
                return [(lambda: qk(mi, qc, 0), kb), (lambda: qk(mi, qc, 1), kb)]

            def VG(tb, kb):
                return [(lambda: vg(tb, 0), kb), (lambda: vg(tb, 1), kb)]

            sched = [
                ((0, 0), VG(0, 0) + VG(1, 0) + VG(2, 1) + VG(3, 2)
                         + VG(4, 3) + VG(5, 4) + VG(6, 4) + VG(7, 5)
                         + QK(1, 0, 5) + QK(CB + 1, 0, 6)),
                ((0, 1), QK(2, 0, 2) + QK(CB + 2, 0, 4) + QK(0, 1, 6)),
                ((0, 2), QK(3, 0, 2) + QK(CB + 3, 0, 4) + QK(CB, 1, 6)),
                ((0, 3), QK(4, 0, 2) + QK(CB + 4, 0, 3) + VG(8, 5) + VG(9, 6)),
                ((0, 4), QK(5, 0, 2) + QK(CB + 5, 0, 3) + VG(10, 4)
                         + VG(11, 5) + QK(1, 1, 6)),
                ((0, 5), QK(CB + 1, 1, 2) + VG(12, 3) + VG(13, 4)
                         + VG(14, 5) + VG(15, 6)),
                ((1, 0), QK(2, 1, 3) + QK(CB + 2, 1, 5)),
                ((1, 1), [(lambda: pj(0, 0, 0), 2), (lambda: pj(0, 0, 1), 2),
                          (lambda: pj(0, 1, 0), 3), (lambda: pj(0, 1, 1), 3)]
                         + QK(3, 1, 5) + QK(CB + 3, 1, 6)),
                ((1, 2), [(lambda: pj(0, 2, 0), 2), (lambda: pj(0, 2, 1), 2),
                          (lambda: pj(0, 3, 0), 3), (lambda: pj(0, 3, 1), 3)]
                         + QK(4, 1, 5) + QK(CB + 4, 1, 6)),
                ((1, 3), [(lambda: pj(0, 4, 0), 2), (lambda: pj(0, 4, 1), 2),
                          (lambda: pj(0, 5, 0), 3), (lambda: pj(0, 5, 1), 3)]),
                ((1, 4), [(lambda: qk(5, 1, 0), 1), (lambda: qk(5, 1, 1), 2),
                          (lambda: qk(CB + 5, 1, 0), 4),
                          (lambda: qk(CB + 5, 1, 1), 5)]),
                ((1, 5), []),
            ]

            pending = None
            for (b, hp), ul in sched:
                units = [u for u, _ in ul]
                unit_kbs = [k for _, k in ul]
                last = (b, hp) == (1, HP - 1)
                pending = emit_attn(b, hp, units, unit_kbs, pending, last)

            # tail: qi=0 proj half-units run while the deferred qi=1
            # epilogue chains normalize on DVE/GpSimd; output DMAs
            # alternate sync/scalar queues to halve the drain
            rest_chains = pending
            qs = [nc.sync, nc.scalar]
            for co in range(CB):
                u_proj(1, co, 0, q=qs[co % 2])
            rest_chains()
            for co in range(CB):
                u_proj(1, co, 1, q=qs[co % 2])

            if _DEBUG:
                nc.sync.dma_start(qk_dbg[:, :, :], qksb[:])
                nc.sync.dma_start(va_dbg[:, :, :, :], v_aug[:])
                nc.sync.dma_start(out_dbg[:, :, :], outT[:])

    nc.finalize()
    return nc


def _get_nc():
    global _CACHED_NC
    if _CACHED_NC is None:
        _CACHED_NC = _build_nc()
    return _CACHED_NC


def _ternary(w):
    """Host-side ternary quantization matching the reference's boundary
    decisions: s/thr in float64, comparisons on the float32 weights."""
    w = np.asarray(w, dtype=np.float32)
    s64 = np.float64(np.mean(np.abs(w), dtype=np.float64))
    s = np.float32(s64)
    thr = np.float32(0.5) * (s + np.float32(EPS))
    t = (w > thr).astype(np.float32) - (w < -thr).astype(np.float32)
    return t, s


def run(x, w_qkv, w_proj, b_proj, trace=False):
    x = np.ascontiguousarray(x, dtype=np.float32)
    tq, s_q = _ternary(w_qkv)    # [3C, C]
    tp, s_p = _ternary(w_proj)   # [C, C]
    bp = np.ascontiguousarray(b_proj, dtype=np.float32)
    es = np.float32(SCALE) * s_q * s_q
    sq = np.array([[s_q, es]], dtype=np.float32)
    sp = np.array([[s_p]], dtype=np.float32)
    cz_host = np.zeros((2, N), dtype=ml_dtypes.bfloat16)
    cz_host[1, :] = 1.0

    tqT = np.ascontiguousarray(tq.T)  # [C, 3C]
    # fp8 DoubleRow packing: [P, pass, kt, cols] flattened per piece
    # (A: Q block 0, B: K block 0, C: Q blocks 1-5, D: K blocks 1-5)
    wq_pkd = tqT[:, : 2 * C].reshape(3, 2, P, 2 * C).transpose(2, 0, 1, 3)
    qpart, kpart = wq_pkd[:, :, :, :C], wq_pkd[:, :, :, C:]
    wq16 = np.ascontiguousarray(np.concatenate(
        [qpart[:, :, :, :P].reshape(P, -1),
         kpart[:, :, :, :P].reshape(P, -1),
         qpart[:, :, :, P:].reshape(P, -1),
         kpart[:, :, :, P:].reshape(P, -1)], axis=1
    )).astype(ml_dtypes.float8_e4m3)
    wv16 = np.ascontiguousarray(
        tqT[:, 2 * C :].reshape(CB, P, C).transpose(1, 0, 2)
    ).astype(ml_dtypes.bfloat16)
    wp16 = np.ascontiguousarray(
        np.ascontiguousarray(tp.T).reshape(CB, P, C).transpose(1, 0, 2)
    ).astype(ml_dtypes.bfloat16)

    in_maps = []
    for c in range(NCORES):
        xs = x[c * BPC : (c + 1) * BPC].reshape(T, C)
        # [P, 2, pass*kt*N]: batch-major, DoubleRow feature-pair packing
        xsT = np.ascontiguousarray(
            xs.T.reshape(3, 2, P, 2, N).transpose(2, 3, 0, 1, 4)
            .reshape(P, 2, -1)
        )
        # [P, 2, CB*N]: bf16 copy for the V path (fp8 V noise alone costs
        # ~2.5e-2 rel error; scores tolerate fp8, V does not)
        xsT16 = np.ascontiguousarray(
            xs.T.reshape(CB, P, 2, N).transpose(1, 2, 0, 3).reshape(P, 2, -1)
        )
        in_maps.append(
            {
                "xT": xsT.astype(ml_dtypes.float8_e4m3),
                "xT16": xsT16.astype(ml_dtypes.bfloat16),
                "wq16": wq16,
                "wv16": wv16,
                "wp16": wp16,
                "bp": bp,
                "sq": sq,
                "sp": sp,
                "cz": cz_host,
            }
        )

    nc = _get_nc()
    res = run_bass_kernel_spmd(
        nc, in_maps, core_ids=list(range(NCORES)), trace=trace
    )

    y = np.empty((B, N, C), dtype=np.float32)
    for c in range(NCORES):
        yT_c = res.results[c]["yT"].reshape(C, T)  # [CB, P, T] -> [C, T]
        y[c * BPC : (c + 1) * BPC] = yT_c.T.reshape(BPC, N, C)
    return y, res


def run_debug(x, w_qkv, w_proj, b_proj):
    global _DEBUG, _CACHED_NC
    _DEBUG = True
    _CACHED_NC = None
    try:
        return run(x, w_qkv, w_proj, b_proj, trace=False)
    finally:
        _DEBUG = False
        _CACHED_NC = None


def kernel(x, w_qkv, w_proj, b_proj):
    y, _ = run(x, w_qkv, w_proj, b_proj, trace=False)
    return y


# revision 44
# speedup vs baseline: 1.0039x; 1.0039x over previous
"""BitNet-style attention block (ternary-quantized QKV/proj) on 8 Trainium2 cores.

Strategy: data-parallel over batch (16 batches -> 2 per core, no collectives).

v5 (462us v3 baseline -> ~390us fast-clock):
  - Q/K generation runs in fp8e4 DoubleRow (K=256 per pass, measured at
    bf16's per-instruction rate -> half the qkgen PE time). Ternary
    weights are exact in fp8; only x quantizes (~3% el-wise), which the
    softmax damps to ~5e-3 output error. V stays bf16 (fp8 V alone
    costs ~2.6e-2 rel error - over the 2e-2 budget).
  - ACT is exp-only plus the 4 per-pair [65,512] PSUM->SBUF AV
    evacuations (its idle boundary window); exp uses an immediate scale
    (Q pre-scaled by SCALE*s^2 at evac), 1079ns/[128,1024] measured.
  - All PSUM evacuations (qk/vgen/proj) on DVE; softmax epilogue per
    (head, query-half): l-row copy -> reciprocal_approx_fast (DVE) ->
    partition_broadcast (GpSimd) -> multiply (DVE). GpSimd runs ONLY
    partition_broadcast steady-state: its other ops live in a different
    Q7 library and each switch costs a ~6us LIBRARY_RELOAD stall.
  - Emission pipeline per kb: ST(kb,0) exp(kb,0) ST(kb,1) exp(kb,1)
    AV(kb-1,1) [epilogue part / filler unit] AV(kb,0). The trailing
    AV(7,1) + avsb evac land at the next pair's kb0 and the 4
    normalize chains spread over its kb1-4 (cross-pair pipelining,
    no DVE head-of-line blocking of the psum-slot-recycling evacs).
  - Filler units (3-6 matmuls + evac) are placed per (pair, kb) by an
    explicit JIT table; kb7 gets a unit so the boundary kb is not
    PE-light. PSUM: 2x st2 [128,1024] + 4x AV accumulators = all 8
    banks; filler units steal exp-freed st2 slots.
  - Inputs land as per-partition-contiguous DMAs (128 descriptors
    each) with separate tiles per piece so the first qkgen waits only
    on its own 0.9MB; priority order x8(b0) / wq blocks 0+6 first.
    Output DMAs alternate sync/scalar queues; proj emits per
    512-token half; the last pair's qi=1 normalize chains overlap the
    tail's qi=0 proj units.
  - Device clock varies run-to-run (~391 vs ~460us for the same NEFF);
    compare kernels by min over a few runs.
"""

import os
import sys

import ml_dtypes
import numpy as np

for _p in ("/opt/trn_rl_repo", "/root/.axon_site/_ro/trn_rl_repo"):
    if os.path.isdir(_p) and _p not in sys.path:
        sys.path.insert(0, _p)

import concourse.bass as bass
import concourse.mybir as mybir
import concourse.tile as tile
from concourse import bacc
from concourse.bass_utils import run_bass_kernel_spmd

B, N, C, H = 16, 1024, 768, 12
HD = C // H                    # 64
SCALE = float(HD ** -0.5)      # 0.125
EPS = 1e-5
NCORES = 8
BPC = B // NCORES              # 2 batches per core
T = BPC * N                    # 2048 tokens per core
P = 128
CB = C // P                    # 6 c-blocks of 128
MQK = 2 * CB                   # 12 d-blocks covering Q and K
HP = H // 2                    # 6 head pairs
KB = N // P                    # 8 key blocks per batch
F32 = mybir.dt.float32
BF16 = mybir.dt.bfloat16
FP8 = mybir.dt.float8e4
DR = mybir.MatmulPerfMode.DoubleRow
AF = mybir.ActivationFunctionType
ALU = mybir.AluOpType

_CACHED_NC = None
_DEBUG = False


def _split_drain_waits(nc):
    """The walrus build in this container accepts only one sync-wait per
    instruction; move extra waits onto preceding single-wait NoOps on the
    same engine (in-order queues make this semantics-preserving)."""
    for fn in nc.m.functions:
        for bb in fn.blocks:
            insts = bb.instructions
            i = 0
            while i < len(insts):
                inst = insts[i]
                si = getattr(inst, "sync_info", None)
                if (
                    si is not None
                    and si.on_wait is not None
                    and len(si.on_wait) > 1
                    and not type(inst).__name__.startswith("InstDMA")
                ):
                    waits = list(si.on_wait)
                    for j, w in enumerate(waits[:-1]):
                        nop = mybir.InstNoOp(
                            name=f"{inst.name}-prewait-{j}", ins=[], outs=[]
                        )
                        nop.engine = inst.engine
                        nop.sync_info = mybir.SyncInfo(on_wait=[w], on_update=[])
                        insts.insert(i, nop)
                        i += 1
                    inst.sync_info = mybir.SyncInfo(
                        on_wait=[waits[-1]], on_update=list(si.on_update)
                    )
                i += 1


def _build_nc():
    nc = bacc.Bacc(None)

    xT = nc.dram_tensor("xT", [P, 2, 6 * N], FP8, kind="ExternalInput")
    xT16 = nc.dram_tensor("xT16", [P, 2, CB * N], BF16, kind="ExternalInput")
    wq16 = nc.dram_tensor("wq16", [P, 6 * 2 * C], FP8, kind="ExternalInput")
    wv16 = nc.dram_tensor("wv16", [P, CB, C], BF16, kind="ExternalInput")
    wp16 = nc.dram_tensor("wp16", [P, CB, C], BF16, kind="ExternalInput")
    bp = nc.dram_tensor("bp", [C], F32, kind="ExternalInput")
    sq = nc.dram_tensor("sq", [1, 2], F32, kind="ExternalInput")  # [s, SCALE*s^2]
    sp = nc.dram_tensor("sp", [1, 1], F32, kind="ExternalInput")  # [s]
    cz = nc.dram_tensor("cz", [2, N], BF16, kind="ExternalInput")  # row0=0, row1=1
    yT = nc.dram_tensor("yT", [CB, P, T], F32, kind="ExternalOutput")
    if _DEBUG:
        qk_dbg = nc.dram_tensor("qk_dbg", [P, MQK, T], BF16, kind="ExternalOutput")
        va_dbg = nc.dram_tensor(
            "va_dbg", [P, 2 * KB, H, HD + 1], BF16, kind="ExternalOutput"
        )
        out_dbg = nc.dram_tensor("out_dbg", [P, CB, T], BF16, kind="ExternalOutput")
        l_dbg = nc.dram_tensor("l_dbg", [4, 512], F32, kind="ExternalOutput")
        li_dbg = nc.dram_tensor("li_dbg", [4, 512], F32, kind="ExternalOutput")
        bc_dbg = nc.dram_tensor("bc_dbg", [4, HD, 512], F32, kind="ExternalOutput")
        am_dbg = nc.dram_tensor("am_dbg", [4, HD, 512], F32, kind="ExternalOutput")

    with tile.TileContext(nc) as tc:
        with (
            tc.tile_pool(name="constp", bufs=1) as constp,
            tc.tile_pool(name="xp", bufs=2) as xp,
            tc.tile_pool(name="wqab", bufs=2) as wqab,
            tc.tile_pool(name="wqcd", bufs=2) as wqcd,
            tc.tile_pool(name="wvp", bufs=1) as wvp,
            tc.tile_pool(name="wpp", bufs=1) as wpp,
            tc.tile_pool(name="vaugp", bufs=1) as vaugp,
            tc.tile_pool(name="qksp", bufs=1) as qksp,
            tc.tile_pool(name="outp", bufs=1) as outp,
            tc.tile_pool(name="ep", bufs=4) as ep,
            tc.tile_pool(name="linvp", bufs=2) as linvp,
            tc.tile_pool(name="bcp", bufs=2) as bcp,
            tc.tile_pool(name="avsp", bufs=4) as avsp,
            tc.tile_pool(name="ystp", bufs=2) as ystp,
            tc.tile_pool(name="psp", bufs=2, space="PSUM") as psp,   # [P,1024] 2-bank
            tc.tile_pool(name="avp", bufs=4, space="PSUM") as avp,   # [65,512] 1-bank
        ):
            # ---- scalars / bias / ones ----
            sqb = constp.tile([P, 2], F32, tag="sqb")
            spb = constp.tile([P, 1], F32, tag="spb")
            b_sb = constp.tile([P, CB], F32, tag="b_sb")
            ones_col = constp.tile([P, 1], BF16, tag="ones_col")

            # ---- inputs, ordered so pair-0 work starts ASAP ----
            # scalar hwdge queue: x by token halves (first half feeds all of
            # batch 0); sync queue: wq d-blocks 0 and 6 (pair-0 Q/K), then
            # the rest; gpsimd queue: wv/wp in parallel.
            # x / wq pieces are separate tiles: dependency domains match
            # DMA granularity exactly (first matmul can't get stuck behind
            # the last weight transfer), and every transfer is one
            # contiguous chunk per partition (128 descriptors, cheap gen).
            # Queue order = priority: scalar queue x0 -> wv -> x1 -> wp
            # (wv feeds the startup vgen units; x1/wp are needed much
            # later), sync queue wq blocks 0/6 first.
            # fp8e4 DoubleRow: partition p carries feature f = pass*256 +
            # kt*128 + p (kt in {0,1}); every qkgen/vgen matmul contracts
            # 256 features per pass at bf16 streaming rate (measured: same
            # ~215ns per 512-free instruction, half the instruction count)
            xs_t = [xp.tile([P, 6 * N], FP8, tag="x8", name=f"x8{q}")
                    for q in range(2)]
            xs16 = [xp.tile([P, CB * N], BF16, tag="x16", name=f"x16{q}")
                    for q in range(2)]
            wq_a = wqab.tile([P, 6 * P], FP8, tag="wqa")
            wq_b = wqab.tile([P, 6 * P], FP8, tag="wqb")
            wq_c = wqcd.tile([P, 6 * (C - P)], FP8, tag="wqc")
            wq_d = wqcd.tile([P, 6 * (C - P)], FP8, tag="wqd")
            wv_q = wvp.tile([P, CB, C], BF16, tag="wv")
            wp_q = wpp.tile([P, CB, C], BF16, tag="wp")
            nc.sync.dma_start(wq_a[:], wq16[:, 0 : 6 * P])
            nc.sync.dma_start(wq_b[:], wq16[:, 6 * P : 12 * P])
            nc.sync.dma_start(sqb[:], sq[:, :].to_broadcast([P, 2]))
            nc.sync.dma_start(ones_col[:], cz[1:2, 0:1].to_broadcast([P, 1]))
            nc.sync.dma_start(spb[:], sp[:, :].to_broadcast([P, 1]))
            nc.sync.dma_start(b_sb[:], bp[:].rearrange("(cb p) -> p cb", p=P))
            nc.scalar.dma_start(xs_t[0][:], xT[:, 0, :])
            nc.scalar.dma_start(xs16[0][:], xT16[:, 0, :])
            nc.scalar.dma_start(wv_q[:].rearrange("p a b -> p (a b)"),
                                wv16[:, :, :].rearrange("p a b -> p (a b)"))
            nc.sync.dma_start(wq_c[:], wq16[:, 12 * P : 12 * P + 6 * (C - P)])
            nc.sync.dma_start(
                wq_d[:], wq16[:, 12 * P + 6 * (C - P) : 12 * C]
            )
            nc.scalar.dma_start(xs_t[1][:], xT[:, 1, :])
            nc.sync.dma_start(xs16[1][:], xT16[:, 1, :])
            nc.sync.dma_start(wp_q[:].rearrange("p a b -> p (a b)"),
                                wp16[:, :, :].rearrange("p a b -> p (a b)"))

            def x16_v(q, ci, lo, hi):
                return xs16[q][:].rearrange("p (c n) -> p c n", c=CB)[:, ci, lo:hi]

            def x_v(q, ps_, lo, hi):
                # [128, 2, hi-lo] rhs/lhsT: (partition, ktile, token)
                v = xs_t[q][:].rearrange("p (a k n) -> p a k n", a=3, k=2)
                return v[:, ps_, :, lo:hi]

            def wq_blk(mi, ps_):
                # [128, 2, 128] lhsT for d-block mi, contraction pass ps_
                if mi == 0:
                    t, off = wq_a, 0
                elif mi == CB:
                    t, off = wq_b, 0
                elif mi < CB:
                    t, off = wq_c, mi - 1
                else:
                    t, off = wq_d, mi - CB - 1
                v = t[:].rearrange("p (a k m) -> p a k m", a=3, k=2)
                return v[:, ps_, :, off * P : (off + 1) * P]

            # ---- V-augmented tile ones column ----
            v_aug = vaugp.tile([P, 2 * KB, H, HD + 1], BF16, tag="vaug")
            nc.vector.tensor_copy(
                v_aug[:, :, :, HD : HD + 1],
                ones_col[:, None, :].to_broadcast([P, 2 * KB, H, 1]),
            )

            qksb = qksp.tile([P, MQK, T], BF16, tag="qksb")
            outT = outp.tile([P, CB, T], BF16, tag="outT")

            # ---------------- filler units (one psp slot each) ----------------

            def u_qkgen(mi, qc, half):
                """Q/K d-block mi, one 512-token half of batch qc (3 fp8
                DoubleRow matmuls, K=256 each). Q blocks (mi<CB) pre-scaled
                by SCALE*s^2 so exp runs with an immediate scale."""
                ps = psp.tile([P, 512], F32, tag="ps", name=f"qg{mi}_{qc}_{half}")
                for ki in range(3):
                    nc.tensor.matmul(
                        ps[:],
                        wq_blk(mi, ki),
                        x_v(qc, ki, half * 512, (half + 1) * 512),
                        start=(ki == 0),
                        stop=(ki == 2),
                        perf_mode=DR,
                    )
                dst = qksb[:, mi, qc * N + half * 512 : qc * N + (half + 1) * 512]
                if mi < CB:
                    nc.vector.tensor_scalar_mul(dst, ps[:], sqb[:, 1:2])
                else:
                    nc.vector.tensor_copy(dst, ps[:])

            def u_vgen(tb, nch):
                """V (scaled by s) for one 128-token block, heads
                nch*6..nch*6+5 (6 bf16 matmuls)."""
                ps = psp.tile([P, 512], F32, tag="ps", name=f"vg{tb}_{nch}")
                for ci in range(CB):
                    nc.tensor.matmul(
                        ps[:, 0:384],
                        x16_v(tb // KB, ci, (tb % KB) * P, (tb % KB + 1) * P),
                        wv_q[:, ci, nch * 384 : (nch + 1) * 384],
                        start=(ci == 0),
                        stop=(ci == CB - 1),
                    )
                nc.vector.tensor_scalar_mul(
                    v_aug[:, tb, nch * 6 : (nch + 1) * 6, 0:HD],
                    ps[:, 0:384].rearrange("p (h d) -> p h d", d=HD),
                    sqb[:, 0:1],
                )

            def u_proj(b, co, half, q=None):
                """proj output block co, one 512-token half of batch b
                (6 matmuls; bias+scale on DVE; result DMA on queue q)."""
                ps = psp.tile([P, 512], F32, tag="ps", name=f"pj{b}_{co}_{half}")
                for ci in range(CB):
                    nc.tensor.matmul(
                        ps[:],
                        wp_q[:, ci, co * P : (co + 1) * P],
                        outT[:, ci, b * N + half * 512 : b * N + (half + 1) * 512],
                        start=(ci == 0),
                        stop=(ci == CB - 1),
                    )
                yst = ystp.tile([P, 512], F32, tag="evac",
                                name=f"ye{b}_{co}_{half}")
                nc.vector.tensor_scalar(
                    yst[:], ps[:], spb[:, 0:1], b_sb[:, co : co + 1],
                    ALU.mult, ALU.add,
                )
                (q or nc.sync).dma_start(
                    yT[co, :, b * N + half * 512 : b * N + (half + 1) * 512],
                    yst[:],
                )

            # ---------------- attention ----------------

            def emit_epilogue_chain(b, hp, hh, qi, srcl, srcm, j):
                """one (hh, qi) softmax-normalize chain: l-row -> partition 0
                (DVE), reciprocal (DVE custom), partition-broadcast (GpSimd,
                the only op class it runs steady-state: keeps the attn
                library resident, no Q7 library reloads), multiply (DVE)."""
                lrow = linvp.tile([1, 512], F32, tag="lrow", name=f"lr{j}")
                nc.vector.tensor_copy(lrow[:], srcl)
                linv = linvp.tile([1, 512], F32, tag="linv", name=f"li{j}")
                nc.vector.reciprocal_approx_fast(linv[:], lrow[:])
                bc = bcp.tile([HD, 512], F32, tag="bc", name=f"bc{j}")
                nc.gpsimd.partition_broadcast(bc[:], linv[0:1, :], channels=HD)
                nc.vector.tensor_mul(
                    out=outT[
                        hh * HD : (hh + 1) * HD,
                        hp,
                        b * N + qi * 512 : b * N + (qi + 1) * 512,
                    ],
                    in0=srcm,
                    in1=bc[:],
                )

            def emit_attn(b, hp, units, unit_kbs, pending, last):
                """attention for batch b, heads 2hp/2hp+1. `units`: filler
                closures; `unit_kbs`: kb step for each. `pending`: list of
                closures finishing the PREVIOUS pair (trailing AV + epilogue
                chains), spread over kb0..kb3 so their DVE work never
                head-of-line-blocks the unit evacs that recycle psum slots.
                Returns this pair's pending list (or emits inline if last)."""
                avs = {}
                e2s = {}

                def emit_st(kb, hh):
                    roff = hh * HD
                    st2 = psp.tile([P, 1024], F32, tag="ps", name=f"st{hh}")
                    for half in range(2):
                        nc.tensor.matmul(
                            st2[:, half * 512 : (half + 1) * 512],
                            qksb[roff : roff + HD, CB + hp,
                                 b * N + kb * P : b * N + (kb + 1) * P],
                            qksb[roff : roff + HD, hp,
                                 b * N + half * 512 : b * N + (half + 1) * 512],
                            start=True,
                            stop=True,
                        )
                    e2 = ep.tile([P, 1024], BF16, tag="e2", name=f"e{hh}")
                    nc.scalar.activation(e2[:], st2[:], AF.Exp, bias=0.0)
                    e2s[(kb, hh)] = e2

                def emit_av(kb, hh):
                    h = 2 * hp + hh
                    e2 = e2s.pop((kb, hh))
                    if kb == 0:
                        # lazy alloc: the previous pair's trailing accesses to
                        # these pool slots must be emitted before the slots
                        # are recycled
                        avs[hh] = [
                            avp.tile(
                                [HD + 1, 512], F32, tag="av", name=f"av{hh}{qi}"
                            )
                            for qi in range(2)
                        ]
                    for qi in range(2):
                        nc.tensor.matmul(
                            avs[hh][qi][:],
                            v_aug[:, b * KB + kb, h, :],
                            e2[:, qi * 512 : (qi + 1) * 512],
                            start=(kb == 0),
                            stop=(kb == KB - 1),
                        )

                ui = 0
                pending = list(pending or [])
                for kb in range(KB):
                    emit_st(kb, 0)
                    emit_st(kb, 1)
                    if kb > 0:
                        emit_av(kb - 1, 1)
                    if kb < len(pending):
                        pending[kb]()
                    while ui < len(units) and unit_kbs[ui] <= kb:
                        units[ui]()
                        ui += 1
                    emit_av(kb, 0)
                while ui < len(units):
                    units[ui]()
                    ui += 1

                # hh0 finished accumulating (stop at kb=KB-1): evacuate its
                # PSUM promptly unless this is the final pair
                if not last:
                    avsb = {}
                    for qi in range(2):
                        t = avsp.tile([HD + 1, 512], F32, tag="avsb",
                                      name=f"ab0{qi}")
                        nc.scalar.activation(t[:], avs[0][qi][:], AF.Copy, bias=0.0)
                        avsb[(0, qi)] = t

                    def part0():
                        emit_av(KB - 1, 1)
                        for qi in range(2):
                            t = avsp.tile([HD + 1, 512], F32, tag="avsb",
                                          name=f"ab1{qi}")
                            nc.scalar.activation(
                                t[:], avs[1][qi][:], AF.Copy, bias=0.0
                            )
                            avsb[(1, qi)] = t

                    def chain(j):
                        hh, qi = j // 2, j % 2
                        emit_epilogue_chain(
                            b, hp, hh, qi,
                            avsb[(hh, qi)][HD : HD + 1, :],
                            avsb[(hh, qi)][0:HD, :],
                            j,
                        )

                    return [part0] + [
                        (lambda j=j: chain(j)) for j in range(4)
                    ]

                # final pair: trailing AV + epilogue straight from PSUM.
                # qi=0 chains now; qi=1 chains deferred so the tail's first
                # proj half-units (which only need qi=0 of outT) overlap them
                emit_av(KB - 1, 1)
                for hh in range(2):
                    emit_epilogue_chain(
                        b, hp, hh, 0,
                        avs[hh][0][HD : HD + 1, :],
                        avs[hh][0][0:HD, :],
                        hh * 2,
                    )

                def rest():
                    for hh in range(2):
                        emit_epilogue_chain(
                            b, hp, hh, 1,
                            avs[hh][1][HD : HD + 1, :],
                            avs[hh][1][0:HD, :],
                            hh * 2 + 1,
                        )

                return rest

            # ---------------- schedule ----------------
            # startup: pair-0 Q/K only; V blocks stream in as pair-0 units
            # (the exp chain starts ~8us earlier than a vgen pre-burst)
            u_qkgen(0, 0, 0)
            u_qkgen(0, 0, 1)
            u_qkgen(CB, 0, 0)
            u_qkgen(CB, 0, 1)

            # per-pair filler unit lists (just-in-time: a pair's Q/K units
            # land >=1 pair ahead; vgen b1 lands >=2 pairs ahead; proj b0
            # waits for the b0 pair-5 epilogue which lands at b1-pair0 kb0)
            qk = u_qkgen
            vg = u_vgen
            pj = u_proj

            def QK(mi, qc, kb):
                return [(lambda: qk(mi, qc, 0), kb), (lambda: qk(mi, qc, 1), kb)]

            def VG(tb, kb):
                return [(lambda: vg(tb, 0), kb), (lambda: vg(tb, 1), kb)]

            sched = [
                ((0, 0), VG(0, 0) + VG(1, 0) + VG(2, 1) + VG(3, 2)
                         + VG(4, 3) + VG(5, 4) + VG(6, 4) + VG(7, 5)
                         + QK(1, 0, 5) + QK(CB + 1, 0, 6)),
                ((0, 1), QK(2, 0, 2) + QK(CB + 2, 0, 4) + QK(0, 1, 6)),
                ((0, 2), QK(3, 0, 2) + QK(CB + 3, 0, 4) + QK(CB, 1, 6)),
                ((0, 3), QK(4, 0, 2) + QK(CB + 4, 0, 3) + VG(8, 5) + VG(9, 6)),
                ((0, 4), QK(5, 0, 2) + QK(CB + 5, 0, 3) + VG(10, 4)
                         + VG(11, 5) + QK(1, 1, 6)),
                ((0, 5), QK(CB + 1, 1, 2) + VG(12, 3) + VG(13, 4)
                         + VG(14, 5) + VG(15, 6)),
                ((1, 0), QK(2, 1, 3) + QK(CB + 2, 1, 5)),
                ((1, 1), [(lambda: pj(0, 0, 0), 2), (lambda: pj(0, 0, 1), 2),
                          (lambda: pj(0, 1, 0), 3), (lambda: pj(0, 1, 1), 3)]
                         + QK(3, 1, 5) + QK(CB + 3, 1, 6)),
                ((1, 2), [(lambda: pj(0, 2, 0), 2), (lambda: pj(0, 2, 1), 2),
                          (lambda: pj(0, 3, 0), 3), (lambda: pj(0, 3, 1), 3)]
                         + QK(4, 1, 5) + QK(CB + 4, 1, 6)),
                ((1, 3), [(lambda: pj(0, 4, 0), 2), (lambda: pj(0, 4, 1), 2),
                          (lambda: pj(0, 5, 0), 3), (lambda: pj(0, 5, 1), 3)]
                         + QK(5, 1, 5) + QK(CB + 5, 1, 6)),
                ((1, 4), []),
                ((1, 5), []),
            ]

            pending = None
            for (b, hp), ul in sched:
                units = [u for u, _ in ul]
                unit_kbs = [k for _, k in ul]
                last = (b, hp) == (1, HP - 1)
                pending = emit_attn(b, hp, units, unit_kbs, pending, last)

            # tail: qi=0 proj half-units run while the deferred qi=1
            # epilogue chains normalize on DVE/GpSimd; output DMAs
            # alternate sync/scalar queues to halve the drain
            rest_chains = pending
            qs = [nc.sync, nc.scalar]
            for co in range(CB):
                u_proj(1, co, 0, q=qs[co % 2])
            rest_chains()
            for co in range(CB):
                u_proj(1, co, 1, q=qs[co % 2])

            if _DEBUG:
                nc.sync.dma_start(qk_dbg[:, :, :], qksb[:])
                nc.sync.dma_start(va_dbg[:, :, :, :], v_aug[:])
                nc.sync.dma_start(out_dbg[:, :, :], outT[:])

    nc.finalize()
    return nc


def _get_nc():
    global _CACHED_NC
    if _CACHED_NC is None:
        _CACHED_NC = _build_nc()
    return _CACHED_NC


def _ternary(w):
    """Host-side ternary quantization matching the reference's boundary
    decisions: s/thr in float64, comparisons on the float32 weights."""
    w = np.asarray(w, dtype=np.float32)
    s64 = np.float64(np.mean(np.abs(w), dtype=np.float64))
    s = np.float32(s64)
    thr = np.float32(0.5) * (s + np.float32(EPS))
    t = (w > thr).astype(np.float32) - (w < -thr).astype(np.float32)
    return t, s


def run(x, w_qkv, w_proj, b_proj, trace=False):
    x = np.ascontiguousarray(x, dtype=np.float32)
    tq, s_q = _ternary(w_qkv)    # [3C, C]
    tp, s_p = _ternary(w_proj)   # [C, C]
    bp = np.ascontiguousarray(b_proj, dtype=np.float32)
    es = np.float32(SCALE) * s_q * s_q
    sq = np.array([[s_q, es]], dtype=np.float32)
    sp = np.array([[s_p]], dtype=np.float32)
    cz_host = np.zeros((2, N), dtype=ml_dtypes.bfloat16)
    cz_host[1, :] = 1.0

    tqT = np.ascontiguousarray(tq.T)  # [C, 3C]
    # fp8 DoubleRow packing: [P, pass, kt, cols] flattened per piece
    # (A: Q block 0, B: K block 0, C: Q blocks 1-5, D: K blocks 1-5)
    wq_pkd = tqT[:, : 2 * C].reshape(3, 2, P, 2 * C).transpose(2, 0, 1, 3)
    qpart, kpart = wq_pkd[:, :, :, :C], wq_pkd[:, :, :, C:]
    wq16 = np.ascontiguousarray(np.concatenate(
        [qpart[:, :, :, :P].reshape(P, -1),
         kpart[:, :, :, :P].reshape(P, -1),
         qpart[:, :, :, P:].reshape(P, -1),
         kpart[:, :, :, P:].reshape(P, -1)], axis=1
    )).astype(ml_dtypes.float8_e4m3)
    wv16 = np.ascontiguousarray(
        tqT[:, 2 * C :].reshape(CB, P, C).transpose(1, 0, 2)
    ).astype(ml_dtypes.bfloat16)
    wp16 = np.ascontiguousarray(
        np.ascontiguousarray(tp.T).reshape(CB, P, C).transpose(1, 0, 2)
    ).astype(ml_dtypes.bfloat16)

    in_maps = []
    for c in range(NCORES):
        xs = x[c * BPC : (c + 1) * BPC].reshape(T, C)
        # [P, 2, pass*kt*N]: batch-major, DoubleRow feature-pair packing
        xsT = np.ascontiguousarray(
            xs.T.reshape(3, 2, P, 2, N).transpose(2, 3, 0, 1, 4)
            .reshape(P, 2, -1)
        )
        # [P, 2, CB*N]: bf16 copy for the V path (fp8 V noise alone costs
        # ~2.5e-2 rel error; scores tolerate fp8, V does not)
        xsT16 = np.ascontiguousarray(
            xs.T.reshape(CB, P, 2, N).transpose(1, 2, 0, 3).reshape(P, 2, -1)
        )
        in_maps.append(
            {
                "xT": xsT.astype(ml_dtypes.float8_e4m3),
                "xT16": xsT16.astype(ml_dtypes.bfloat16),
                "wq16": wq16,
                "wv16": wv16,
                "wp16": wp16,
                "bp": bp,
                "sq": sq,
                "sp": sp,
                "cz": cz_host,
            }
        )

    nc = _get_nc()
    res = run_bass_kernel_spmd(
        nc, in_maps, core_ids=list(range(NCORES)), trace=trace
    )

    y = np.empty((B, N, C), dtype=np.float32)
    for c in range(NCORES):
        yT_c = res.results[c]["yT"].reshape(C, T)  # [CB, P, T] -> [C, T]
        y[c * BPC : (c + 1) * BPC] = yT_c.T.reshape(BPC, N, C)
    return y, res


def run_debug(x, w_qkv, w_proj, b_proj):
    global _DEBUG, _CACHED_NC
    _DEBUG = True
    _CACHED_NC = None
    try:
        return run(x, w_qkv, w_proj, b_proj, trace=False)
    finally:
        _DEBUG = False
        _CACHED_NC = None


def kernel(x, w_qkv, w_proj, b_proj):
    y, _ = run(x, w_qkv, w_proj, b_proj, trace=False)
    return y


# revision 46
# speedup vs baseline: 1.0065x; 1.0025x over previous
"""BitNet-style attention block (ternary-quantized QKV/proj) on 8 Trainium2 cores.

Strategy: data-parallel over batch (16 batches -> 2 per core, no collectives).

v5 (462us v3 baseline -> ~390us fast-clock):
  - Q/K generation runs in fp8e4 DoubleRow (K=256 per pass, measured at
    bf16's per-instruction rate -> half the qkgen PE time). Ternary
    weights are exact in fp8; only x quantizes (~3% el-wise), which the
    softmax damps to ~5e-3 output error. V stays bf16 (fp8 V alone
    costs ~2.6e-2 rel error - over the 2e-2 budget).
  - ACT is exp-only plus the 4 per-pair [65,512] PSUM->SBUF AV
    evacuations (its idle boundary window); exp uses an immediate scale
    (Q pre-scaled by SCALE*s^2 at evac), 1079ns/[128,1024] measured.
  - All PSUM evacuations (qk/vgen/proj) on DVE; softmax epilogue per
    (head, query-half): l-row copy -> reciprocal_approx_fast (DVE) ->
    partition_broadcast (GpSimd) -> multiply (DVE). GpSimd runs ONLY
    partition_broadcast steady-state: its other ops live in a different
    Q7 library and each switch costs a ~6us LIBRARY_RELOAD stall.
  - Emission pipeline per kb: ST(kb,0) exp(kb,0) ST(kb,1) exp(kb,1)
    AV(kb-1,1) [epilogue part / filler unit] AV(kb,0). The trailing
    AV(7,1) + avsb evac land at the next pair's kb0 and the 4
    normalize chains spread over its kb1-4 (cross-pair pipelining,
    no DVE head-of-line blocking of the psum-slot-recycling evacs).
  - Filler units (3-6 matmuls + evac) are placed per (pair, kb) by an
    explicit JIT table; kb7 gets a unit so the boundary kb is not
    PE-light. PSUM: 2x st2 [128,1024] + 4x AV accumulators = all 8
    banks; filler units steal exp-freed st2 slots.
  - Inputs land as per-partition-contiguous DMAs (128 descriptors
    each) with separate tiles per piece so the first qkgen waits only
    on its own 0.9MB; priority order x8(b0) / wq blocks 0+6 first.
    Output DMAs alternate sync/scalar queues; proj emits per
    512-token half; the last pair's qi=1 normalize chains overlap the
    tail's qi=0 proj units.
  - Device clock varies run-to-run (~391 vs ~460us for the same NEFF);
    compare kernels by min over a few runs.
"""

import os
import sys

import ml_dtypes
import numpy as np

for _p in ("/opt/trn_rl_repo", "/root/.axon_site/_ro/trn_rl_repo"):
    if os.path.isdir(_p) and _p not in sys.path:
        sys.path.insert(0, _p)

import concourse.bass as bass
import concourse.mybir as mybir
import concourse.tile as tile
from concourse import bacc
from concourse.bass_utils import run_bass_kernel_spmd

B, N, C, H = 16, 1024, 768, 12
HD = C // H                    # 64
SCALE = float(HD ** -0.5)      # 0.125
EPS = 1e-5
NCORES = 8
BPC = B // NCORES              # 2 batches per core
T = BPC * N                    # 2048 tokens per core
P = 128
CB = C // P                    # 6 c-blocks of 128
MQK = 2 * CB                   # 12 d-blocks covering Q and K
HP = H // 2                    # 6 head pairs
KB = N // P                    # 8 key blocks per batch
F32 = mybir.dt.float32
BF16 = mybir.dt.bfloat16
FP8 = mybir.dt.float8e4
DR = mybir.MatmulPerfMode.DoubleRow
AF = mybir.ActivationFunctionType
ALU = mybir.AluOpType

_CACHED_NC = None
_DEBUG = False


def _split_drain_waits(nc):
    """The walrus build in this container accepts only one sync-wait per
    instruction; move extra waits onto preceding single-wait NoOps on the
    same engine (in-order queues make this semantics-preserving)."""
    for fn in nc.m.functions:
        for bb in fn.blocks:
            insts = bb.instructions
            i = 0
            while i < len(insts):
                inst = insts[i]
                si = getattr(inst, "sync_info", None)
                if (
                    si is not None
                    and si.on_wait is not None
                    and len(si.on_wait) > 1
                    and not type(inst).__name__.startswith("InstDMA")
                ):
                    waits = list(si.on_wait)
                    for j, w in enumerate(waits[:-1]):
                        nop = mybir.InstNoOp(
                            name=f"{inst.name}-prewait-{j}", ins=[], outs=[]
                        )
                        nop.engine = inst.engine
                        nop.sync_info = mybir.SyncInfo(on_wait=[w], on_update=[])
                        insts.insert(i, nop)
                        i += 1
                    inst.sync_info = mybir.SyncInfo(
                        on_wait=[waits[-1]], on_update=list(si.on_update)
                    )
                i += 1


def _build_nc():
    nc = bacc.Bacc(None)

    xT = nc.dram_tensor("xT", [P, 2, 6 * N], FP8, kind="ExternalInput")
    xT16 = nc.dram_tensor("xT16", [P, 2, CB * N], BF16, kind="ExternalInput")
    wq16 = nc.dram_tensor("wq16", [P, 6 * 2 * C], FP8, kind="ExternalInput")
    wv16 = nc.dram_tensor("wv16", [P, CB, C], BF16, kind="ExternalInput")
    wp16 = nc.dram_tensor("wp16", [P, CB, C], BF16, kind="ExternalInput")
    bp = nc.dram_tensor("bp", [C], F32, kind="ExternalInput")
    sq = nc.dram_tensor("sq", [1, 2], F32, kind="ExternalInput")  # [s, SCALE*s^2]
    sp = nc.dram_tensor("sp", [1, 1], F32, kind="ExternalInput")  # [s]
    cz = nc.dram_tensor("cz", [2, N], BF16, kind="ExternalInput")  # row0=0, row1=1
    yT = nc.dram_tensor("yT", [CB, P, T], F32, kind="ExternalOutput")
    if _DEBUG:
        qk_dbg = nc.dram_tensor("qk_dbg", [P, MQK, T], BF16, kind="ExternalOutput")
        va_dbg = nc.dram_tensor(
            "va_dbg", [P, 2 * KB, H, HD + 1], BF16, kind="ExternalOutput"
        )
        out_dbg = nc.dram_tensor("out_dbg", [P, CB, T], BF16, kind="ExternalOutput")
        l_dbg = nc.dram_tensor("l_dbg", [4, 512], F32, kind="ExternalOutput")
        li_dbg = nc.dram_tensor("li_dbg", [4, 512], F32, kind="ExternalOutput")
        bc_dbg = nc.dram_tensor("bc_dbg", [4, HD, 512], F32, kind="ExternalOutput")
        am_dbg = nc.dram_tensor("am_dbg", [4, HD, 512], F32, kind="ExternalOutput")

    with tile.TileContext(nc) as tc:
        with (
            tc.tile_pool(name="constp", bufs=1) as constp,
            tc.tile_pool(name="xp", bufs=2) as xp,
            tc.tile_pool(name="wqab", bufs=2) as wqab,
            tc.tile_pool(name="wqcd", bufs=2) as wqcd,
            tc.tile_pool(name="wvp", bufs=1) as wvp,
            tc.tile_pool(name="wpp", bufs=1) as wpp,
            tc.tile_pool(name="vaugp", bufs=1) as vaugp,
            tc.tile_pool(name="qksp", bufs=1) as qksp,
            tc.tile_pool(name="outp", bufs=1) as outp,
            tc.tile_pool(name="ep", bufs=4) as ep,
            tc.tile_pool(name="linvp", bufs=2) as linvp,
            tc.tile_pool(name="bcp", bufs=2) as bcp,
            tc.tile_pool(name="avsp", bufs=4) as avsp,
            tc.tile_pool(name="ystp", bufs=2) as ystp,
            tc.tile_pool(name="psp", bufs=2, space="PSUM") as psp,   # [P,1024] 2-bank
            tc.tile_pool(name="avp", bufs=4, space="PSUM") as avp,   # [65,512] 1-bank
        ):
            # ---- scalars / bias / ones ----
            sqb = constp.tile([P, 2], F32, tag="sqb")
            spb = constp.tile([P, 1], F32, tag="spb")
            b_sb = constp.tile([P, CB], F32, tag="b_sb")
            ones_col = constp.tile([P, 1], BF16, tag="ones_col")

            # ---- inputs, ordered so pair-0 work starts ASAP ----
            # scalar hwdge queue: x by token halves (first half feeds all of
            # batch 0); sync queue: wq d-blocks 0 and 6 (pair-0 Q/K), then
            # the rest; gpsimd queue: wv/wp in parallel.
            # x / wq pieces are separate tiles: dependency domains match
            # DMA granularity exactly (first matmul can't get stuck behind
            # the last weight transfer), and every transfer is one
            # contiguous chunk per partition (128 descriptors, cheap gen).
            # Queue order = priority: scalar queue x0 -> wv -> x1 -> wp
            # (wv feeds the startup vgen units; x1/wp are needed much
            # later), sync queue wq blocks 0/6 first.
            # fp8e4 DoubleRow: partition p carries feature f = pass*256 +
            # kt*128 + p (kt in {0,1}); every qkgen/vgen matmul contracts
            # 256 features per pass at bf16 streaming rate (measured: same
            # ~215ns per 512-free instruction, half the instruction count)
            xs_t = [xp.tile([P, 6 * N], FP8, tag="x8", name=f"x8{q}")
                    for q in range(2)]
            xs16 = [xp.tile([P, CB * N], BF16, tag="x16", name=f"x16{q}")
                    for q in range(2)]
            wq_a = wqab.tile([P, 6 * P], FP8, tag="wqa")
            wq_b = wqab.tile([P, 6 * P], FP8, tag="wqb")
            wq_c = wqcd.tile([P, 6 * (C - P)], FP8, tag="wqc")
            wq_d = wqcd.tile([P, 6 * (C - P)], FP8, tag="wqd")
            wv_q = wvp.tile([P, CB, C], BF16, tag="wv")
            wp_q = wpp.tile([P, CB, C], BF16, tag="wp")
            nc.sync.dma_start(wq_a[:], wq16[:, 0 : 6 * P])
            nc.sync.dma_start(wq_b[:], wq16[:, 6 * P : 12 * P])
            nc.sync.dma_start(sqb[:], sq[:, :].to_broadcast([P, 2]))
            nc.sync.dma_start(ones_col[:], cz[1:2, 0:1].to_broadcast([P, 1]))
            nc.sync.dma_start(spb[:], sp[:, :].to_broadcast([P, 1]))
            nc.sync.dma_start(b_sb[:], bp[:].rearrange("(cb p) -> p cb", p=P))
            nc.scalar.dma_start(xs_t[0][:], xT[:, 0, :])
            nc.scalar.dma_start(xs16[0][:], xT16[:, 0, :])
            nc.scalar.dma_start(wv_q[:].rearrange("p a b -> p (a b)"),
                                wv16[:, :, :].rearrange("p a b -> p (a b)"))
            nc.sync.dma_start(wq_c[:], wq16[:, 12 * P : 12 * P + 6 * (C - P)])
            nc.sync.dma_start(
                wq_d[:], wq16[:, 12 * P + 6 * (C - P) : 12 * C]
            )
            nc.scalar.dma_start(xs_t[1][:], xT[:, 1, :])
            nc.sync.dma_start(xs16[1][:], xT16[:, 1, :])
            nc.sync.dma_start(wp_q[:].rearrange("p a b -> p (a b)"),
                                wp16[:, :, :].rearrange("p a b -> p (a b)"))

            def x16_v(q, ci, lo, hi):
                return xs16[q][:].rearrange("p (c n) -> p c n", c=CB)[:, ci, lo:hi]

            def x_v(q, ps_, lo, hi):
                # [128, 2, hi-lo] rhs/lhsT: (partition, ktile, token)
                v = xs_t[q][:].rearrange("p (a k n) -> p a k n", a=3, k=2)
                return v[:, ps_, :, lo:hi]

            def wq_blk(mi, ps_):
                # [128, 2, 128] lhsT for d-block mi, contraction pass ps_
                if mi == 0:
                    t, off = wq_a, 0
                elif mi == CB:
                    t, off = wq_b, 0
                elif mi < CB:
                    t, off = wq_c, mi - 1
                else:
                    t, off = wq_d, mi - CB - 1
                v = t[:].rearrange("p (a k m) -> p a k m", a=3, k=2)
                return v[:, ps_, :, off * P : (off + 1) * P]

            # ---- V-augmented tile ones column ----
            v_aug = vaugp.tile([P, 2 * KB, H, HD + 1], BF16, tag="vaug")
            nc.vector.tensor_copy(
                v_aug[:, :, :, HD : HD + 1],
                ones_col[:, None, :].to_broadcast([P, 2 * KB, H, 1]),
            )

            qksb = qksp.tile([P, MQK, T], BF16, tag="qksb")
            outT = outp.tile([P, CB, T], BF16, tag="outT")

            # ---------------- filler units (one psp slot each) ----------------

            def u_qkgen(mi, qc, half):
                """Q/K d-block mi, one 512-token half of batch qc (3 fp8
                DoubleRow matmuls, K=256 each). Q blocks (mi<CB) pre-scaled
                by SCALE*s^2 so exp runs with an immediate scale."""
                ps = psp.tile([P, 512], F32, tag="ps", name=f"qg{mi}_{qc}_{half}")
                for ki in range(3):
                    nc.tensor.matmul(
                        ps[:],
                        wq_blk(mi, ki),
                        x_v(qc, ki, half * 512, (half + 1) * 512),
                        start=(ki == 0),
                        stop=(ki == 2),
                        perf_mode=DR,
                    )
                dst = qksb[:, mi, qc * N + half * 512 : qc * N + (half + 1) * 512]
                if mi < CB:
                    nc.vector.tensor_scalar_mul(dst, ps[:], sqb[:, 1:2])
                else:
                    nc.vector.tensor_copy(dst, ps[:])

            def u_vgen(tb, nch):
                """V (scaled by s) for one 128-token block, heads
                nch*6..nch*6+5 (6 bf16 matmuls)."""
                ps = psp.tile([P, 512], F32, tag="ps", name=f"vg{tb}_{nch}")
                for ci in range(CB):
                    nc.tensor.matmul(
                        ps[:, 0:384],
                        x16_v(tb // KB, ci, (tb % KB) * P, (tb % KB + 1) * P),
                        wv_q[:, ci, nch * 384 : (nch + 1) * 384],
                        start=(ci == 0),
                        stop=(ci == CB - 1),
                    )
                nc.vector.tensor_scalar_mul(
                    v_aug[:, tb, nch * 6 : (nch + 1) * 6, 0:HD],
                    ps[:, 0:384].rearrange("p (h d) -> p h d", d=HD),
                    sqb[:, 0:1],
                )

            def u_proj(b, co, half, q=None):
                """proj output block co, one 512-token half of batch b
                (6 matmuls; bias+scale on DVE; result DMA on queue q)."""
                ps = psp.tile([P, 512], F32, tag="ps", name=f"pj{b}_{co}_{half}")
                for ci in range(CB):
                    nc.tensor.matmul(
                        ps[:],
                        wp_q[:, ci, co * P : (co + 1) * P],
                        outT[:, ci, b * N + half * 512 : b * N + (half + 1) * 512],
                        start=(ci == 0),
                        stop=(ci == CB - 1),
                    )
                yst = ystp.tile([P, 512], F32, tag="evac",
                                name=f"ye{b}_{co}_{half}")
                nc.vector.tensor_scalar(
                    yst[:], ps[:], spb[:, 0:1], b_sb[:, co : co + 1],
                    ALU.mult, ALU.add,
                )
                (q or nc.sync).dma_start(
                    yT[co, :, b * N + half * 512 : b * N + (half + 1) * 512],
                    yst[:],
                )

            # ---------------- attention ----------------

            def emit_epilogue_chain(b, hp, hh, qi, srcl, srcm, j):
                """one (hh, qi) softmax-normalize chain: l-row -> partition 0
                (DVE), reciprocal (DVE custom), partition-broadcast (GpSimd,
                the only op class it runs steady-state: keeps the attn
                library resident, no Q7 library reloads), multiply (DVE)."""
                lrow = linvp.tile([1, 512], F32, tag="lrow", name=f"lr{j}")
                nc.vector.tensor_copy(lrow[:], srcl)
                linv = linvp.tile([1, 512], F32, tag="linv", name=f"li{j}")
                nc.vector.reciprocal_approx_fast(linv[:], lrow[:])
                bc = bcp.tile([HD, 512], F32, tag="bc", name=f"bc{j}")
                nc.gpsimd.partition_broadcast(bc[:], linv[0:1, :], channels=HD)
                nc.vector.tensor_mul(
                    out=outT[
                        hh * HD : (hh + 1) * HD,
                        hp,
                        b * N + qi * 512 : b * N + (qi + 1) * 512,
                    ],
                    in0=srcm,
                    in1=bc[:],
                )

            def emit_attn(b, hp, units, unit_kbs, pending, last):
                """attention for batch b, heads 2hp/2hp+1. `units`: filler
                closures; `unit_kbs`: kb step for each. `pending`: list of
                closures finishing the PREVIOUS pair (trailing AV + epilogue
                chains), spread over kb0..kb3 so their DVE work never
                head-of-line-blocks the unit evacs that recycle psum slots.
                Returns this pair's pending list (or emits inline if last)."""
                avs = {}
                e2s = {}

                def emit_st(kb, hh):
                    roff = hh * HD
                    st2 = psp.tile([P, 1024], F32, tag="ps", name=f"st{hh}")
                    for half in range(2):
                        nc.tensor.matmul(
                            st2[:, half * 512 : (half + 1) * 512],
                            qksb[roff : roff + HD, CB + hp,
                                 b * N + kb * P : b * N + (kb + 1) * P],
                            qksb[roff : roff + HD, hp,
                                 b * N + half * 512 : b * N + (half + 1) * 512],
                            start=True,
                            stop=True,
                        )
                    e2 = ep.tile([P, 1024], BF16, tag="e2", name=f"e{hh}")
                    nc.scalar.activation(e2[:], st2[:], AF.Exp, bias=0.0)
                    e2s[(kb, hh)] = e2

                def emit_av(kb, hh):
                    h = 2 * hp + hh
                    e2 = e2s.pop((kb, hh))
                    if kb == 0:
                        # lazy alloc: the previous pair's trailing accesses to
                        # these pool slots must be emitted before the slots
                        # are recycled
                        avs[hh] = [
                            avp.tile(
                                [HD + 1, 512], F32, tag="av", name=f"av{hh}{qi}"
                            )
                            for qi in range(2)
                        ]
                    for qi in range(2):
                        nc.tensor.matmul(
                            avs[hh][qi][:],
                            v_aug[:, b * KB + kb, h, :],
                            e2[:, qi * 512 : (qi + 1) * 512],
                            start=(kb == 0),
                            stop=(kb == KB - 1),
                        )

                ui = 0
                pending = list(pending or [])
                for kb in range(KB):
                    emit_st(kb, 0)
                    emit_st(kb, 1)
                    if kb > 0:
                        emit_av(kb - 1, 1)
                    if kb < len(pending):
                        pending[kb]()
                    while ui < len(units) and unit_kbs[ui] <= kb:
                        units[ui]()
                        ui += 1
                    emit_av(kb, 0)
                while ui < len(units):
                    units[ui]()
                    ui += 1

                # hh0 finished accumulating (stop at kb=KB-1): evacuate its
                # PSUM promptly unless this is the final pair
                if not last:
                    avsb = {}
                    for qi in range(2):
                        t = avsp.tile([HD + 1, 512], F32, tag="avsb",
                                      name=f"ab0{qi}")
                        nc.scalar.activation(t[:], avs[0][qi][:], AF.Copy, bias=0.0)
                        avsb[(0, qi)] = t

                    def part0():
                        emit_av(KB - 1, 1)
                        for qi in range(2):
                            t = avsp.tile([HD + 1, 512], F32, tag="avsb",
                                          name=f"ab1{qi}")
                            nc.scalar.activation(
                                t[:], avs[1][qi][:], AF.Copy, bias=0.0
                            )
                            avsb[(1, qi)] = t

                    def chain(j):
                        hh, qi = j // 2, j % 2
                        emit_epilogue_chain(
                            b, hp, hh, qi,
                            avsb[(hh, qi)][HD : HD + 1, :],
                            avsb[(hh, qi)][0:HD, :],
                            j,
                        )

                    return [part0] + [
                        (lambda j=j: chain(j)) for j in range(4)
                    ]

                # final pair: trailing AV + epilogue straight from PSUM.
                # qi=0 chains now; qi=1 chains deferred so the tail's first
                # proj half-units (which only need qi=0 of outT) overlap them
                emit_av(KB - 1, 1)
                for hh in range(2):
                    emit_epilogue_chain(
                        b, hp, hh, 0,
                        avs[hh][0][HD : HD + 1, :],
                        avs[hh][0][0:HD, :],
                        hh * 2,
                    )

                def rest():
                    for hh in range(2):
                        emit_epilogue_chain(
                            b, hp, hh, 1,
                            avs[hh][1][HD : HD + 1, :],
                            avs[hh][1][0:HD, :],
                            hh * 2 + 1,
                        )

                return rest

            # ---------------- schedule ----------------
            # startup: pair-0 Q/K only; V blocks stream in as pair-0 units
            # (the exp chain starts ~8us earlier than a vgen pre-burst)
            u_qkgen(0, 0, 0)
            u_qkgen(0, 0, 1)
            u_qkgen(CB, 0, 0)
            u_qkgen(CB, 0, 1)

            # per-pair filler unit lists (just-in-time: a pair's Q/K units
            # land >=1 pair ahead; vgen b1 lands >=2 pairs ahead; proj b0
            # waits for the b0 pair-5 epilogue which lands at b1-pair0 kb0)
            qk = u_qkgen
            vg = u_vgen
            pj = u_proj

            def QK(mi, qc, kb):
                return [(lambda: qk(mi, qc, 0), kb), (lambda: qk(mi, qc, 1), kb)]

            def VG(tb, kb):
                return [(lambda: vg(tb, 0), kb), (lambda: vg(tb, 1), kb)]

            sched = [
                ((0, 0), VG(0, 0) + VG(1, 0) + VG(2, 1) + VG(3, 2)
                         + VG(4, 3) + VG(5, 4) + VG(6, 4) + VG(7, 5)
                         + QK(1, 0, 5) + QK(CB + 1, 0, 6)),
                ((0, 1), QK(2, 0, 2) + QK(CB + 2, 0, 4) + QK(0, 1, 6)),
                ((0, 2), QK(3, 0, 2) + QK(CB + 3, 0, 4) + QK(CB, 1, 6)),
                ((0, 3), QK(4, 0, 2) + QK(CB + 4, 0, 3) + VG(8, 5) + VG(9, 6)),
                ((0, 4), QK(5, 0, 2) + QK(CB + 5, 0, 3) + VG(10, 4)
                         + VG(11, 5) + QK(1, 1, 6)),
                ((0, 5), QK(CB + 1, 1, 2) + VG(12, 3) + VG(13, 4)
                         + VG(14, 5) + VG(15, 6)),
                ((1, 0), QK(2, 1, 3) + QK(CB + 2, 1, 5)),
                ((1, 1), [(lambda: pj(0, 0, 0), 2), (lambda: pj(0, 0, 1), 2),
                          (lambda: pj(0, 1, 0), 3), (lambda: pj(0, 1, 1), 3)]
                         + QK(3, 1, 5) + QK(CB + 3, 1, 6)),
                ((1, 2), [(lambda: pj(0, 2, 0), 2), (lambda: pj(0, 2, 1), 2),
                          (lambda: pj(0, 3, 0), 3), (lambda: pj(0, 3, 1), 3)]
                         + QK(4, 1, 5) + QK(CB + 4, 1, 6)),
                ((1, 3), [(lambda: pj(0, 4, 0), 2), (lambda: pj(0, 4, 1), 2),
                          (lambda: pj(0, 5, 0), 3), (lambda: pj(0, 5, 1), 3)]
                         + QK(5, 1, 5) + QK(CB + 5, 1, 6)),
                ((1, 4), []),
                ((1, 5), []),
            ]

            pending = None
            for (b, hp), ul in sched:
                units = [u for u, _ in ul]
                unit_kbs = [k for _, k in ul]
                last = (b, hp) == (1, HP - 1)
                pending = emit_attn(b, hp, units, unit_kbs, pending, last)

            # tail: qi=0 proj half-units run while the deferred qi=1
            # epilogue chains normalize on DVE/GpSimd; output DMAs
            # alternate sync/scalar queues to halve the drain
            rest_chains = pending
            qs = [nc.sync, nc.scalar]
            for co in range(CB):
                u_proj(1, co, 0, q=qs[co % 2])
            rest_chains()
            for co in range(CB):
                u_proj(1, co, 1, q=qs[co % 2])

            if _DEBUG:
                nc.sync.dma_start(qk_dbg[:, :, :], qksb[:])
                nc.sync.dma_start(va_dbg[:, :, :, :], v_aug[:])
                nc.sync.dma_start(out_dbg[:, :, :], outT[:])

    nc.finalize()
    return nc


def _get_nc():
    global _CACHED_NC
    if _CACHED_NC is None:
        _CACHED_NC = _build_nc()
    return _CACHED_NC


def _ternary(w):
    """Host-side ternary quantization matching the reference's boundary
    decisions: s/thr in float64, comparisons on the float32 weights."""
    w = np.asarray(w, dtype=np.float32)
    s64 = np.float64(np.mean(np.abs(w), dtype=np.float64))
    s = np.float32(s64)
    thr = np.float32(0.5) * (s + np.float32(EPS))
    t = (w > thr).astype(np.float32) - (w < -thr).astype(np.float32)
    return t, s


def run(x, w_qkv, w_proj, b_proj, trace=False):
    x = np.ascontiguousarray(x, dtype=np.float32)
    tq, s_q = _ternary(w_qkv)    # [3C, C]
    tp, s_p = _ternary(w_proj)   # [C, C]
    bp = np.ascontiguousarray(b_proj, dtype=np.float32)
    es = np.float32(SCALE) * s_q * s_q
    sq = np.array([[s_q, es]], dtype=np.float32)
    sp = np.array([[s_p]], dtype=np.float32)
    cz_host = np.zeros((2, N), dtype=ml_dtypes.bfloat16)
    cz_host[1, :] = 1.0

    tqT = np.ascontiguousarray(tq.T)  # [C, 3C]
    # fp8 DoubleRow packing: [P, pass, kt, cols] flattened per piece
    # (A: Q block 0, B: K block 0, C: Q blocks 1-5, D: K blocks 1-5)
    wq_pkd = tqT[:, : 2 * C].reshape(3, 2, P, 2 * C).transpose(2, 0, 1, 3)
    qpart, kpart = wq_pkd[:, :, :, :C], wq_pkd[:, :, :, C:]
    wq16 = np.ascontiguousarray(np.concatenate(
        [qpart[:, :, :, :P].reshape(P, -1),
         kpart[:, :, :, :P].reshape(P, -1),
         qpart[:, :, :, P:].reshape(P, -1),
         kpart[:, :, :, P:].reshape(P, -1)], axis=1
    )).astype(ml_dtypes.float8_e4m3)
    wv16 = np.ascontiguousarray(
        tqT[:, 2 * C :].reshape(CB, P, C).transpose(1, 0, 2)
    ).astype(ml_dtypes.bfloat16)
    wp16 = np.ascontiguousarray(
        np.ascontiguousarray(tp.T).reshape(CB, P, C).transpose(1, 0, 2)
    ).astype(ml_dtypes.bfloat16)

    in_maps = []
    for c in range(NCORES):
        xs = x[c * BPC : (c + 1) * BPC].reshape(T, C)
        # [P, 2, pass*kt*N]: batch-major, DoubleRow feature-pair packing
        xsT = np.ascontiguousarray(
            xs.T.reshape(3, 2, P, 2, N).transpose(2, 3, 0, 1, 4)
            .reshape(P, 2, -1)
        )
        # [P, 2, CB*N]: bf16 copy for the V path (fp8 V noise alone costs
        # ~2.5e-2 rel error; scores tolerate fp8, V does not)
        xsT16 = np.ascontiguousarray(
            xs.T.reshape(CB, P, 2, N).transpose(1, 2, 0, 3).reshape(P, 2, -1)
        )
        in_maps.append(
            {
                "xT": xsT.astype(ml_dtypes.float8_e4m3),
                "xT16": xsT16.astype(ml_dtypes.bfloat16),
                "wq16": wq16,
                "wv16": wv16,
                "wp16": wp16,
                "bp": bp,
                "sq": sq,
                "sp": sp,
                "cz": cz_host,
            }
        )

    nc = _get_nc()
    res = run_bass_kernel_spmd(
        nc, in_maps, core_ids=list(range(NCORES)), trace=trace
    )

    y = np.empty((B, N, C), dtype=np.float32)
    for c in range(NCORES):
        yT_c = res.results[c]["yT"].reshape(C, T)  # [CB, P, T] -> [C, T]
        y[c * BPC : (c + 1) * BPC] = yT_c.T.reshape(BPC, N, C)
    return y, res


def run_debug(x, w_qkv, w_proj, b_proj):
    global _DEBUG, _CACHED_NC
    _DEBUG = True
    _CACHED_NC = None
    try:
        return run(x, w_qkv, w_proj, b_proj, trace=False)
    finally:
        _DEBUG = False
        _CACHED_NC = None


def kernel(x, w_qkv, w_proj, b_proj):
    y, _ = run(x, w_qkv, w_proj, b_proj, trace=False)
    return y


# revision 47
# speedup vs baseline: 1.0212x; 1.0146x over previous
"""BitNet-style attention block (ternary-quantized QKV/proj) on 8 Trainium2 cores.

Strategy: data-parallel over batch (16 batches -> 2 per core, no collectives).

v5 (462us v3 baseline -> ~390us fast-clock):
  - Q/K generation runs in fp8e4 DoubleRow (K=256 per pass, measured at
    bf16's per-instruction rate -> half the qkgen PE time). Ternary
    weights are exact in fp8; only x quantizes (~3% el-wise), which the
    softmax damps to ~5e-3 output error. V stays bf16 (fp8 V alone
    costs ~2.6e-2 rel error - over the 2e-2 budget).
  - ACT is exp-only plus the 4 per-pair [65,512] PSUM->SBUF AV
    evacuations (its idle boundary window); exp uses an immediate scale
    (Q pre-scaled by SCALE*s^2 at evac), 1079ns/[128,1024] measured.
  - All PSUM evacuations (qk/vgen/proj) on DVE; softmax epilogue per
    (head, query-half): l-row copy -> reciprocal_approx_fast (DVE) ->
    partition_broadcast (GpSimd) -> multiply (DVE). GpSimd runs ONLY
    partition_broadcast steady-state: its other ops live in a different
    Q7 library and each switch costs a ~6us LIBRARY_RELOAD stall.
  - Emission pipeline per kb: ST(kb,0) exp(kb,0) ST(kb,1) exp(kb,1)
    AV(kb-1,1) [epilogue part / filler unit] AV(kb,0). The trailing
    AV(7,1) + avsb evac land at the next pair's kb0 and the 4
    normalize chains spread over its kb1-4 (cross-pair pipelining,
    no DVE head-of-line blocking of the psum-slot-recycling evacs).
  - Filler units (3-6 matmuls + evac) are placed per (pair, kb) by an
    explicit JIT table; kb7 gets a unit so the boundary kb is not
    PE-light. PSUM: 2x st2 [128,1024] + 4x AV accumulators = all 8
    banks; filler units steal exp-freed st2 slots.
  - Inputs land as per-partition-contiguous DMAs (128 descriptors
    each) with separate tiles per piece so the first qkgen waits only
    on its own 0.9MB; priority order x8(b0) / wq blocks 0+6 first.
    Output DMAs alternate sync/scalar queues; proj emits per
    512-token half; the last pair's qi=1 normalize chains overlap the
    tail's qi=0 proj units.
  - Device clock varies run-to-run (~391 vs ~460us for the same NEFF);
    compare kernels by min over a few runs.
"""

import os
import sys

import ml_dtypes
import numpy as np

for _p in ("/opt/trn_rl_repo", "/root/.axon_site/_ro/trn_rl_repo"):
    if os.path.isdir(_p) and _p not in sys.path:
        sys.path.insert(0, _p)

import concourse.bass as bass
import concourse.mybir as mybir
import concourse.tile as tile
from concourse import bacc
from concourse.bass_utils import run_bass_kernel_spmd

B, N, C, H = 16, 1024, 768, 12
HD = C // H                    # 64
SCALE = float(HD ** -0.5)      # 0.125
EPS = 1e-5
NCORES = 8
BPC = B // NCORES              # 2 batches per core
T = BPC * N                    # 2048 tokens per core
P = 128
CB = C // P                    # 6 c-blocks of 128
MQK = 2 * CB                   # 12 d-blocks covering Q and K
HP = H // 2                    # 6 head pairs
KB = N // P                    # 8 key blocks per batch
F32 = mybir.dt.float32
BF16 = mybir.dt.bfloat16
FP8 = mybir.dt.float8e4
DR = mybir.MatmulPerfMode.DoubleRow
AF = mybir.ActivationFunctionType
ALU = mybir.AluOpType

_CACHED_NC = None
_DEBUG = False


def _split_drain_waits(nc):
    """The walrus build in this container accepts only one sync-wait per
    instruction; move extra waits onto preceding single-wait NoOps on the
    same engine (in-order queues make this semantics-preserving)."""
    for fn in nc.m.functions:
        for bb in fn.blocks:
            insts = bb.instructions
            i = 0
            while i < len(insts):
                inst = insts[i]
                si = getattr(inst, "sync_info", None)
                if (
                    si is not None
                    and si.on_wait is not None
                    and len(si.on_wait) > 1
                    and not type(inst).__name__.startswith("InstDMA")
                ):
                    waits = list(si.on_wait)
                    for j, w in enumerate(waits[:-1]):
                        nop = mybir.InstNoOp(
                            name=f"{inst.name}-prewait-{j}", ins=[], outs=[]
                        )
                        nop.engine = inst.engine
                        nop.sync_info = mybir.SyncInfo(on_wait=[w], on_update=[])
                        insts.insert(i, nop)
                        i += 1
                    inst.sync_info = mybir.SyncInfo(
                        on_wait=[waits[-1]], on_update=list(si.on_update)
                    )
                i += 1


def _build_nc():
    nc = bacc.Bacc(None)

    xT = nc.dram_tensor("xT", [P, 2, 6 * N], FP8, kind="ExternalInput")
    xT16 = nc.dram_tensor("xT16", [P, 2, CB * N], BF16, kind="ExternalInput")
    wq16 = nc.dram_tensor("wq16", [P, 6 * 2 * C], FP8, kind="ExternalInput")
    wv16 = nc.dram_tensor("wv16", [P, CB, C], BF16, kind="ExternalInput")
    wp16 = nc.dram_tensor("wp16", [P, CB, C], BF16, kind="ExternalInput")
    bp = nc.dram_tensor("bp", [C], F32, kind="ExternalInput")
    sq = nc.dram_tensor("sq", [1, 2], F32, kind="ExternalInput")  # [s, SCALE*s^2]
    sp = nc.dram_tensor("sp", [1, 1], F32, kind="ExternalInput")  # [s]
    cz = nc.dram_tensor("cz", [2, N], BF16, kind="ExternalInput")  # row0=0, row1=1
    yT = nc.dram_tensor("yT", [CB, P, T], F32, kind="ExternalOutput")
    if _DEBUG:
        qk_dbg = nc.dram_tensor("qk_dbg", [P, MQK, T], BF16, kind="ExternalOutput")
        va_dbg = nc.dram_tensor(
            "va_dbg", [P, 2 * KB, H, HD + 1], BF16, kind="ExternalOutput"
        )
        out_dbg = nc.dram_tensor("out_dbg", [P, CB, T], BF16, kind="ExternalOutput")
        l_dbg = nc.dram_tensor("l_dbg", [4, 512], F32, kind="ExternalOutput")
        li_dbg = nc.dram_tensor("li_dbg", [4, 512], F32, kind="ExternalOutput")
        bc_dbg = nc.dram_tensor("bc_dbg", [4, HD, 512], F32, kind="ExternalOutput")
        am_dbg = nc.dram_tensor("am_dbg", [4, HD, 512], F32, kind="ExternalOutput")

    with tile.TileContext(nc) as tc:
        with (
            tc.tile_pool(name="constp", bufs=1) as constp,
            tc.tile_pool(name="xp", bufs=2) as xp,
            tc.tile_pool(name="wqab", bufs=2) as wqab,
            tc.tile_pool(name="wqcd", bufs=2) as wqcd,
            tc.tile_pool(name="wvp", bufs=1) as wvp,
            tc.tile_pool(name="wpp", bufs=1) as wpp,
            tc.tile_pool(name="vaugp", bufs=1) as vaugp,
            tc.tile_pool(name="qksp", bufs=1) as qksp,
            tc.tile_pool(name="outp", bufs=1) as outp,
            tc.tile_pool(name="ep", bufs=4) as ep,
            tc.tile_pool(name="linvp", bufs=2) as linvp,
            tc.tile_pool(name="bcp", bufs=2) as bcp,
            tc.tile_pool(name="avsp", bufs=4) as avsp,
            tc.tile_pool(name="ystp", bufs=2) as ystp,
            tc.tile_pool(name="psp", bufs=2, space="PSUM") as psp,   # [P,1024] 2-bank
            tc.tile_pool(name="avp", bufs=4, space="PSUM") as avp,   # [65,512] 1-bank
        ):
            # ---- scalars / bias / ones ----
            sqb = constp.tile([P, 2], F32, tag="sqb")
            spb = constp.tile([P, 1], F32, tag="spb")
            b_sb = constp.tile([P, CB], F32, tag="b_sb")
            ones_col = constp.tile([P, 1], BF16, tag="ones_col")

            # ---- inputs, ordered so pair-0 work starts ASAP ----
            # scalar hwdge queue: x by token halves (first half feeds all of
            # batch 0); sync queue: wq d-blocks 0 and 6 (pair-0 Q/K), then
            # the rest; gpsimd queue: wv/wp in parallel.
            # x / wq pieces are separate tiles: dependency domains match
            # DMA granularity exactly (first matmul can't get stuck behind
            # the last weight transfer), and every transfer is one
            # contiguous chunk per partition (128 descriptors, cheap gen).
            # Queue order = priority: scalar queue x0 -> wv -> x1 -> wp
            # (wv feeds the startup vgen units; x1/wp are needed much
            # later), sync queue wq blocks 0/6 first.
            # fp8e4 DoubleRow: partition p carries feature f = pass*256 +
            # kt*128 + p (kt in {0,1}); every qkgen/vgen matmul contracts
            # 256 features per pass at bf16 streaming rate (measured: same
            # ~215ns per 512-free instruction, half the instruction count)
            xs_t = [xp.tile([P, 6 * N], FP8, tag="x8", name=f"x8{q}")
                    for q in range(2)]
            xs16 = [xp.tile([P, CB * N], BF16, tag="x16", name=f"x16{q}")
                    for q in range(2)]
            wq_a = wqab.tile([P, 6 * P], FP8, tag="wqa")
            wq_b = wqab.tile([P, 6 * P], FP8, tag="wqb")
            wq_c = wqcd.tile([P, 6 * (C - P)], FP8, tag="wqc")
            wq_d = wqcd.tile([P, 6 * (C - P)], FP8, tag="wqd")
            wv_q = wvp.tile([P, CB, C], BF16, tag="wv")
            wp_q = wpp.tile([P, CB, C], BF16, tag="wp")
            nc.sync.dma_start(wq_a[:], wq16[:, 0 : 6 * P])
            nc.sync.dma_start(wq_b[:], wq16[:, 6 * P : 12 * P])
            nc.sync.dma_start(sqb[:], sq[:, :].to_broadcast([P, 2]))
            nc.sync.dma_start(ones_col[:], cz[1:2, 0:1].to_broadcast([P, 1]))
            nc.sync.dma_start(spb[:], sp[:, :].to_broadcast([P, 1]))
            nc.sync.dma_start(b_sb[:], bp[:].rearrange("(cb p) -> p cb", p=P))
            nc.scalar.dma_start(xs_t[0][:, 0 : 3 * N], xT[:, 0, 0 : 3 * N])
            nc.scalar.dma_start(xs_t[0][:, 3 * N : 6 * N], xT[:, 0, 3 * N :])
            nc.scalar.dma_start(xs16[0][:], xT16[:, 0, :])
            nc.scalar.dma_start(wv_q[:].rearrange("p a b -> p (a b)"),
                                wv16[:, :, :].rearrange("p a b -> p (a b)"))
            nc.sync.dma_start(wq_c[:], wq16[:, 12 * P : 12 * P + 6 * (C - P)])
            nc.sync.dma_start(
                wq_d[:], wq16[:, 12 * P + 6 * (C - P) : 12 * C]
            )
            nc.scalar.dma_start(xs_t[1][:], xT[:, 1, :])
            nc.sync.dma_start(xs16[1][:], xT16[:, 1, :])
            nc.sync.dma_start(wp_q[:].rearrange("p a b -> p (a b)"),
                                wp16[:, :, :].rearrange("p a b -> p (a b)"))

            def x16_v(q, ci, lo, hi):
                return xs16[q][:].rearrange("p (c n) -> p c n", c=CB)[:, ci, lo:hi]

            def x_v(q, ps_, lo, hi):
                # [128, 2, 512] rhs: (partition, ktile, token); the fp8 x is
                # half-major so the startup DMA can deliver the first
                # 512-token half alone (all calls are half-aligned)
                assert lo % 512 == 0 and hi == lo + 512
                v = xs_t[q][:].rearrange(
                    "p (h a k n) -> p h a k n", h=2, a=3, k=2
                )
                return v[:, lo // 512, ps_, :, :]

            def wq_blk(mi, ps_):
                # [128, 2, 128] lhsT for d-block mi, contraction pass ps_
                if mi == 0:
                    t, off = wq_a, 0
                elif mi == CB:
                    t, off = wq_b, 0
                elif mi < CB:
                    t, off = wq_c, mi - 1
                else:
                    t, off = wq_d, mi - CB - 1
                v = t[:].rearrange("p (a k m) -> p a k m", a=3, k=2)
                return v[:, ps_, :, off * P : (off + 1) * P]

            # ---- V-augmented tile ones column ----
            v_aug = vaugp.tile([P, 2 * KB, H, HD + 1], BF16, tag="vaug")
            nc.vector.tensor_copy(
                v_aug[:, :, :, HD : HD + 1],
                ones_col[:, None, :].to_broadcast([P, 2 * KB, H, 1]),
            )

            qksb = qksp.tile([P, MQK, T], BF16, tag="qksb")
            outT = outp.tile([P, CB, T], BF16, tag="outT")

            # ---------------- filler units (one psp slot each) ----------------

            def u_qkgen(mi, qc, half):
                """Q/K d-block mi, one 512-token half of batch qc (3 fp8
                DoubleRow matmuls, K=256 each). Q blocks (mi<CB) pre-scaled
                by SCALE*s^2 so exp runs with an immediate scale."""
                ps = psp.tile([P, 512], F32, tag="ps", name=f"qg{mi}_{qc}_{half}")
                for ki in range(3):
                    nc.tensor.matmul(
                        ps[:],
                        wq_blk(mi, ki),
                        x_v(qc, ki, half * 512, (half + 1) * 512),
                        start=(ki == 0),
                        stop=(ki == 2),
                        perf_mode=DR,
                    )
                dst = qksb[:, mi, qc * N + half * 512 : qc * N + (half + 1) * 512]
                if mi < CB:
                    nc.vector.tensor_scalar_mul(dst, ps[:], sqb[:, 1:2])
                else:
                    nc.vector.tensor_copy(dst, ps[:])

            def u_vgen(tb, nch):
                """V (scaled by s) for one 128-token block, heads
                nch*6..nch*6+5 (6 bf16 matmuls)."""
                ps = psp.tile([P, 512], F32, tag="ps", name=f"vg{tb}_{nch}")
                for ci in range(CB):
                    nc.tensor.matmul(
                        ps[:, 0:384],
                        x16_v(tb // KB, ci, (tb % KB) * P, (tb % KB + 1) * P),
                        wv_q[:, ci, nch * 384 : (nch + 1) * 384],
                        start=(ci == 0),
                        stop=(ci == CB - 1),
                    )
                nc.vector.tensor_scalar_mul(
                    v_aug[:, tb, nch * 6 : (nch + 1) * 6, 0:HD],
                    ps[:, 0:384].rearrange("p (h d) -> p h d", d=HD),
                    sqb[:, 0:1],
                )

            def u_proj(b, co, half, q=None):
                """proj output block co, one 512-token half of batch b
                (6 matmuls; bias+scale on DVE; result DMA on queue q)."""
                ps = psp.tile([P, 512], F32, tag="ps", name=f"pj{b}_{co}_{half}")
                for ci in range(CB):
                    nc.tensor.matmul(
                        ps[:],
                        wp_q[:, ci, co * P : (co + 1) * P],
                        outT[:, ci, b * N + half * 512 : b * N + (half + 1) * 512],
                        start=(ci == 0),
                        stop=(ci == CB - 1),
                    )
                yst = ystp.tile([P, 512], F32, tag="evac",
                                name=f"ye{b}_{co}_{half}")
                nc.vector.tensor_scalar(
                    yst[:], ps[:], spb[:, 0:1], b_sb[:, co : co + 1],
                    ALU.mult, ALU.add,
                )
                (q or nc.sync).dma_start(
                    yT[co, :, b * N + half * 512 : b * N + (half + 1) * 512],
                    yst[:],
                )

            # ---------------- attention ----------------

            def emit_epilogue_chain(b, hp, hh, qi, srcl, srcm, j):
                """one (hh, qi) softmax-normalize chain: l-row -> partition 0
                (DVE), reciprocal (DVE custom), partition-broadcast (GpSimd,
                the only op class it runs steady-state: keeps the attn
                library resident, no Q7 library reloads), multiply (DVE)."""
                lrow = linvp.tile([1, 512], F32, tag="lrow", name=f"lr{j}")
                nc.vector.tensor_copy(lrow[:], srcl)
                linv = linvp.tile([1, 512], F32, tag="linv", name=f"li{j}")
                nc.vector.reciprocal_approx_fast(linv[:], lrow[:])
                bc = bcp.tile([HD, 512], F32, tag="bc", name=f"bc{j}")
                nc.gpsimd.partition_broadcast(bc[:], linv[0:1, :], channels=HD)
                nc.vector.tensor_mul(
                    out=outT[
                        hh * HD : (hh + 1) * HD,
                        hp,
                        b * N + qi * 512 : b * N + (qi + 1) * 512,
                    ],
                    in0=srcm,
                    in1=bc[:],
                )

            def emit_attn(b, hp, units, unit_kbs, pending, last):
                """attention for batch b, heads 2hp/2hp+1. `units`: filler
                closures; `unit_kbs`: kb step for each. `pending`: list of
                closures finishing the PREVIOUS pair (trailing AV + epilogue
                chains), spread over kb0..kb3 so their DVE work never
                head-of-line-blocks the unit evacs that recycle psum slots.
                Returns this pair's pending list (or emits inline if last)."""
                avs = {}
                e2s = {}

                def emit_st(kb, hh):
                    roff = hh * HD
                    st2 = psp.tile([P, 1024], F32, tag="ps", name=f"st{hh}")
                    for half in range(2):
                        nc.tensor.matmul(
                            st2[:, half * 512 : (half + 1) * 512],
                            qksb[roff : roff + HD, CB + hp,
                                 b * N + kb * P : b * N + (kb + 1) * P],
                            qksb[roff : roff + HD, hp,
                                 b * N + half * 512 : b * N + (half + 1) * 512],
                            start=True,
                            stop=True,
                        )
                    e2 = ep.tile([P, 1024], BF16, tag="e2", name=f"e{hh}")
                    nc.scalar.activation(e2[:], st2[:], AF.Exp, bias=0.0)
                    e2s[(kb, hh)] = e2

                def emit_av(kb, hh):
                    h = 2 * hp + hh
                    e2 = e2s.pop((kb, hh))
                    if kb == 0:
                        # lazy alloc: the previous pair's trailing accesses to
                        # these pool slots must be emitted before the slots
                        # are recycled
                        avs[hh] = [
                            avp.tile(
                                [HD + 1, 512], F32, tag="av", name=f"av{hh}{qi}"
                            )
                            for qi in range(2)
                        ]
                    for qi in range(2):
                        nc.tensor.matmul(
                            avs[hh][qi][:],
                            v_aug[:, b * KB + kb, h, :],
                            e2[:, qi * 512 : (qi + 1) * 512],
                            start=(kb == 0),
                            stop=(kb == KB - 1),
                        )

                ui = 0
                pending = list(pending or [])
                for kb in range(KB):
                    emit_st(kb, 0)
                    emit_st(kb, 1)
                    if kb > 0:
                        emit_av(kb - 1, 1)
                    if kb < len(pending):
                        pending[kb]()
                    while ui < len(units) and unit_kbs[ui] <= kb:
                        units[ui]()
                        ui += 1
                    emit_av(kb, 0)
                while ui < len(units):
                    units[ui]()
                    ui += 1

                # hh0 finished accumulating (stop at kb=KB-1): evacuate its
                # PSUM promptly unless this is the final pair
                if not last:
                    avsb = {}
                    for qi in range(2):
                        t = avsp.tile([HD + 1, 512], F32, tag="avsb",
                                      name=f"ab0{qi}")
                        nc.scalar.activation(t[:], avs[0][qi][:], AF.Copy, bias=0.0)
                        avsb[(0, qi)] = t

                    def part0():
                        emit_av(KB - 1, 1)
                        for qi in range(2):
                            t = avsp.tile([HD + 1, 512], F32, tag="avsb",
                                          name=f"ab1{qi}")
                            nc.scalar.activation(
                                t[:], avs[1][qi][:], AF.Copy, bias=0.0
                            )
                            avsb[(1, qi)] = t

                    def chain(j):
                        hh, qi = j // 2, j % 2
                        emit_epilogue_chain(
                            b, hp, hh, qi,
                            avsb[(hh, qi)][HD : HD + 1, :],
                            avsb[(hh, qi)][0:HD, :],
                            j,
                        )

                    return [part0] + [
                        (lambda j=j: chain(j)) for j in range(4)
                    ]

                # final pair: trailing AV + epilogue straight from PSUM.
                # qi=0 chains now; qi=1 chains deferred so the tail's first
                # proj half-units (which only need qi=0 of outT) overlap them
                emit_av(KB - 1, 1)
                for hh in range(2):
                    emit_epilogue_chain(
                        b, hp, hh, 0,
                        avs[hh][0][HD : HD + 1, :],
                        avs[hh][0][0:HD, :],
                        hh * 2,
                    )

                def rest():
                    for hh in range(2):
                        emit_epilogue_chain(
                            b, hp, hh, 1,
                            avs[hh][1][HD : HD + 1, :],
                            avs[hh][1][0:HD, :],
                            hh * 2 + 1,
                        )

                return rest

            # ---------------- schedule ----------------
            # startup: pair-0 Q/K only; V blocks stream in as pair-0 units
            # (the exp chain starts ~8us earlier than a vgen pre-burst)
            u_qkgen(0, 0, 0)
            u_qkgen(0, 0, 1)
            u_qkgen(CB, 0, 0)
            u_qkgen(CB, 0, 1)

            # per-pair filler unit lists (just-in-time: a pair's Q/K units
            # land >=1 pair ahead; vgen b1 lands >=2 pairs ahead; proj b0
            # waits for the b0 pair-5 epilogue which lands at b1-pair0 kb0)
            qk = u_qkgen
            vg = u_vgen
            pj = u_proj

            def QK(mi, qc, kb):
                return [(lambda: qk(mi, qc, 0), kb), (lambda: qk(mi, qc, 1), kb)]

            def VG(tb, kb):
                return [(lambda: vg(tb, 0), kb), (lambda: vg(tb, 1), kb)]

            sched = [
                ((0, 0), VG(0, 0) + VG(1, 0) + VG(2, 1) + VG(3, 2)
                         + VG(4, 3) + VG(5, 4) + VG(6, 4) + VG(7, 5)
                         + QK(1, 0, 5) + QK(CB + 1, 0, 6)),
                ((0, 1), QK(2, 0, 2) + QK(CB + 2, 0, 4) + QK(0, 1, 6)),
                ((0, 2), QK(3, 0, 2) + QK(CB + 3, 0, 4) + QK(CB, 1, 6)),
                ((0, 3), QK(4, 0, 2) + QK(CB + 4, 0, 3) + VG(8, 5) + VG(9, 6)),
                ((0, 4), QK(5, 0, 2) + QK(CB + 5, 0, 3) + VG(10, 4)
                         + VG(11, 5) + QK(1, 1, 6)),
                ((0, 5), QK(CB + 1, 1, 2) + VG(12, 3) + VG(13, 4)
                         + VG(14, 5) + VG(15, 6)),
                ((1, 0), QK(2, 1, 3) + QK(CB + 2, 1, 5)),
                ((1, 1), [(lambda: pj(0, 0, 0), 2), (lambda: pj(0, 0, 1), 2),
                          (lambda: pj(0, 1, 0), 3), (lambda: pj(0, 1, 1), 3)]
                         + QK(3, 1, 5) + QK(CB + 3, 1, 6)),
                ((1, 2), [(lambda: pj(0, 2, 0), 2), (lambda: pj(0, 2, 1), 2),
                          (lambda: pj(0, 3, 0), 3), (lambda: pj(0, 3, 1), 3)]
                         + QK(4, 1, 5) + QK(CB + 4, 1, 6)),
                ((1, 3), [(lambda: pj(0, 4, 0), 2), (lambda: pj(0, 4, 1), 2),
                          (lambda: pj(0, 5, 0), 3), (lambda: pj(0, 5, 1), 3)]
                         + QK(5, 1, 5) + QK(CB + 5, 1, 6)),
                ((1, 4), []),
                ((1, 5), []),
            ]

            pending = None
            for (b, hp), ul in sched:
                units = [u for u, _ in ul]
                unit_kbs = [k for _, k in ul]
                last = (b, hp) == (1, HP - 1)
                pending = emit_attn(b, hp, units, unit_kbs, pending, last)

            # tail: qi=0 proj half-units run while the deferred qi=1
            # epilogue chains normalize on DVE/GpSimd; output DMAs
            # alternate sync/scalar queues to halve the drain
            rest_chains = pending
            qs = [nc.sync, nc.scalar]
            for co in range(CB):
                u_proj(1, co, 0, q=qs[co % 2])
            rest_chains()
            for co in range(CB):
                u_proj(1, co, 1, q=qs[co % 2])

            if _DEBUG:
                nc.sync.dma_start(qk_dbg[:, :, :], qksb[:])
                nc.sync.dma_start(va_dbg[:, :, :, :], v_aug[:])
                nc.sync.dma_start(out_dbg[:, :, :], outT[:])

    nc.finalize()
    return nc


def _get_nc():
    global _CACHED_NC
    if _CACHED_NC is None:
        _CACHED_NC = _build_nc()
    return _CACHED_NC


def _ternary(w):
    """Host-side ternary quantization matching the reference's boundary
    decisions: s/thr in float64, comparisons on the float32 weights."""
    w = np.asarray(w, dtype=np.float32)
    s64 = np.float64(np.mean(np.abs(w), dtype=np.float64))
    s = np.float32(s64)
    thr = np.float32(0.5) * (s + np.float32(EPS))
    t = (w > thr).astype(np.float32) - (w < -thr).astype(np.float32)
    return t, s


def run(x, w_qkv, w_proj, b_proj, trace=False):
    x = np.ascontiguousarray(x, dtype=np.float32)
    tq, s_q = _ternary(w_qkv)    # [3C, C]
    tp, s_p = _ternary(w_proj)   # [C, C]
    bp = np.ascontiguousarray(b_proj, dtype=np.float32)
    es = np.float32(SCALE) * s_q * s_q
    sq = np.array([[s_q, es]], dtype=np.float32)
    sp = np.array([[s_p]], dtype=np.float32)
    cz_host = np.zeros((2, N), dtype=ml_dtypes.bfloat16)
    cz_host[1, :] = 1.0

    tqT = np.ascontiguousarray(tq.T)  # [C, 3C]
    # fp8 DoubleRow packing: [P, pass, kt, cols] flattened per piece
    # (A: Q block 0, B: K block 0, C: Q blocks 1-5, D: K blocks 1-5)
    wq_pkd = tqT[:, : 2 * C].reshape(3, 2, P, 2 * C).transpose(2, 0, 1, 3)
    qpart, kpart = wq_pkd[:, :, :, :C], wq_pkd[:, :, :, C:]
    wq16 = np.ascontiguousarray(np.concatenate(
        [qpart[:, :, :, :P].reshape(P, -1),
         kpart[:, :, :, :P].reshape(P, -1),
         qpart[:, :, :, P:].reshape(P, -1),
         kpart[:, :, :, P:].reshape(P, -1)], axis=1
    )).astype(ml_dtypes.float8_e4m3)
    wv16 = np.ascontiguousarray(
        tqT[:, 2 * C :].reshape(CB, P, C).transpose(1, 0, 2)
    ).astype(ml_dtypes.bfloat16)
    wp16 = np.ascontiguousarray(
        np.ascontiguousarray(tp.T).reshape(CB, P, C).transpose(1, 0, 2)
    ).astype(ml_dtypes.bfloat16)

    in_maps = []
    for c in range(NCORES):
        xs = x[c * BPC : (c + 1) * BPC].reshape(T, C)
        # [P, 2, pass*kt*N]: batch-major, DoubleRow feature-pair packing
        xsT = np.ascontiguousarray(
            xs.T.reshape(3, 2, P, 2, 2, 512).transpose(2, 3, 4, 0, 1, 5)
            .reshape(P, 2, -1)
        )
        # [P, 2, CB*N]: bf16 copy for the V path (fp8 V noise alone costs
        # ~2.5e-2 rel error; scores tolerate fp8, V does not)
        xsT16 = np.ascontiguousarray(
            xs.T.reshape(CB, P, 2, N).transpose(1, 2, 0, 3).reshape(P, 2, -1)
        )
        in_maps.append(
            {
                "xT": xsT.astype(ml_dtypes.float8_e4m3),
                "xT16": xsT16.astype(ml_dtypes.bfloat16),
                "wq16": wq16,
                "wv16": wv16,
                "wp16": wp16,
                "bp": bp,
                "sq": sq,
                "sp": sp,
                "cz": cz_host,
            }
        )

    nc = _get_nc()
    res = run_bass_kernel_spmd(
        nc, in_maps, core_ids=list(range(NCORES)), trace=trace
    )

    y = np.empty((B, N, C), dtype=np.float32)
    for c in range(NCORES):
        yT_c = res.results[c]["yT"].reshape(C, T)  # [CB, P, T] -> [C, T]
        y[c * BPC : (c + 1) * BPC] = yT_c.T.reshape(BPC, N, C)
    return y, res


def run_debug(x, w_qkv, w_proj, b_proj):
    global _DEBUG, _CACHED_NC
    _DEBUG = True
    _CACHED_NC = None
    try:
        return run(x, w_qkv, w_proj, b_proj, trace=False)
    finally:
        _DEBUG = False
        _CACHED_NC = None


def kernel(x, w_qkv, w_proj, b_proj):
    y, _ = run(x, w_qkv, w_proj, b_proj, trace=False)
    return y
